# revision 1
# baseline (speedup 1.0000x reference)
"""GCN message-passing kernel (nn_CARM_90185723281482) for 8 Trainium2 cores.

Computes, for x [2048, 64, 512], adj_weight [64, 64], kernel [512, 64]:
    adj_hat = D^-1/2 A D^-1/2 + I          (degree from row sums of |A|)
    out = BN(elu(adj_hat @ (x @ kernel) + bias))        -> [2048, 64, 64]

Sharding: data-parallel over the batch axis, 256 batches per core.
Per-core dataflow (rows n = (batch, channel) flattened, R = 16384 rows):
  - x ships bf16, host-pre-transposed: xs[p, j, n] = x2d[n, 128 j + p]
  - load block LB = 2048 rows (4 MiB per 2-LB DMA); first LB split small so
    the pipeline starts early
  - stage 1: supportT[d, n] += kern_j.T @ xT_j into one [128, 512] PSUM tile
    (both 512-row halves of a pb stacked on partitions), single drain
  - PE-transpose supportT back to support chunks ssb [n, (gl,t,d)]
  - stage 2 TRANSPOSED: zT[f, n] = ssb_chunk.T @ a2t per 128-wide f-chunk,
    so d = partition % 64 — all BN/bias constants become per-partition
    scalars riding the ACT bias/scale ports and TensorScalarPtr operands
  - 3-op epilogue (a = gamma*rsqrt(var+eps) folded into the stage-1 kernel
    when a > 0, so z_a = a*z comes off the PE; y = z + bias):
        q = exp(inv_a*z_a + bias + ln a)  = a*exp(y)           [ACT]
        t = min(q, a) + (b2 - a)                               [DVE ts]
        out = max(z_a + a*bias + b2, t)                        [DVE stt]
    which equals a*elu(y) + b2 on both branches (y <= e^y - 1 makes the
    max select the relu branch exactly when y >= 0).
  - output stored transposed [f, n]; host un-permutes.

Tuned against the TimelineSim cost model: 69428 ns (baseline) -> 64106 ns.
HW-verified rel err 3.7e-3.
"""

import sys

import numpy as np

sys.path.insert(0, "/opt/trn_rl_repo")

import concourse.bass as bass  # noqa: E402
from concourse import bacc, bass_utils, mybir, tile  # noqa: E402

F32 = mybir.dt.float32
BF16 = mybir.dt.bfloat16
AF = mybir.ActivationFunctionType
OP = mybir.AluOpType

NCORES = 8
B_FULL, C, Fdim, D = 2048, 64, 512, 64
R = (B_FULL // NCORES) * C  # 16384 rows per core
LB_ROWS = 2048              # rows per load block
NLB = R // LB_ROWS          # 8 load blocks
BN_EPS = 1e-3

_NC_CACHE = {}

# Scheduling/balance knobs (tuned against the TimelineSim cost model)
CFG = {
    "px": 6,
    "psT_sb": 6,
    "ps_sb": 3,
    "pep": 4,
    "psT_ps": 3,
    "ps_ps": 2,
    "po_ps": 3,
    "sT_split": 1,         # stage-1 PSUM: 0 = one [128,512], 1 = two [64,512]
    "ssb_split": 0,        # support drain: 0 = whole, 1 = per 256-col half
    "sT_engine": ["act", "dve"],   # supportT drain engine (per gl)
    "ssb_engine": ["dve", "act"],  # support drain engine (per pb)
    "r_engine": "act",     # relu (safe variant): "act" | "dve" | "pool"
    "t_engine": "dve",     # min/add tensor_scalar
    "add_engine": "dve",   # final scalar_tensor_tensor / add
    "store_lbs": 4,
    "tailsplit": 1,
    "load_lbs": 1,
    "split_last_store": 1,
    "warmup_mm": 0,        # dummy matmuls (reading cstb) to ramp the PE
    "warmup_act": 0,       # dummy Exp to preload the ACT table early
    "head_pieces": (1024,),
    "ep_split": 0,         # epilogue per 256-col half
    "direct_s1": 1,        # stage-1 with x stationary: no transposes/drains
    "d1_drainw": 512,      # direct stage-1 drain width (128|256|512)
}


def _pick(v, pb):
    """Engine knob: either a name or a [pb0, pb1] alternation list."""
    return v[pb % len(v)] if isinstance(v, (list, tuple)) else v


def to_bf16(a):
    """fp32 -> bf16 (RNE), returned as a uint16 array (raw bf16 bits)."""
    u = np.ascontiguousarray(a, np.float32).view(np.uint32).astype(np.uint64)
    r = (u + 0x7FFF + ((u >> 16) & 1)) >> 16
    return r.astype(np.uint16)


def _build_nc(loop_reps=None, variant="ln"):
    nc = bacc.Bacc(
        "TRN2", target_bir_lowering=False, debug=False, num_devices=NCORES
    )
    xs_d = nc.dram_tensor("xs", [Fdim, R], BF16, kind="ExternalInput").ap()
    cstb_d = nc.dram_tensor("cstb", [128, 512], BF16, kind="ExternalInput").ap()
    cst2_d = nc.dram_tensor("cst2", [128, 8], F32, kind="ExternalInput").ap()
    out_d = nc.dram_tensor("out", [128, (R // 128) * D], BF16,
                           kind="ExternalOutput").ap()

    with tile.TileContext(nc) as tc, \
         tc.tile_pool(name="consts", bufs=1) as consts, \
         tc.tile_pool(name="px", bufs=CFG["px"]) as px, \
         tc.tile_pool(name="psT_ps", bufs=CFG["psT_ps"], space="PSUM") as psT_ps, \
         tc.tile_pool(name="psT_sb", bufs=CFG["psT_sb"]) as psT_sb, \
         tc.tile_pool(name="ps_ps", bufs=CFG["ps_ps"], space="PSUM") as ps_ps, \
         tc.tile_pool(name="ps_sb", bufs=CFG["ps_sb"]) as ps_sb, \
         tc.tile_pool(name="po_ps", bufs=CFG["po_ps"], space="PSUM") as po_ps, \
         tc.tile_pool(name="pep", bufs=CFG["pep"]) as pep, \
         tc.tile_pool(name="pout", bufs=2) as pout:

        cstb = consts.tile([128, 512], BF16, tag="cstb")
        nc.sync.dma_start(cstb[:], cstb_d)
        cst2 = consts.tile([128, 8], F32, tag="cst2")
        identr = cstb[:, 0:128]
        kern = cstb[:, 128:384]
        a2t = cstb[:, 384:512]
        inv_a = cst2[:, 0:1]
        bias_exp = cst2[:, 1:2]
        rbias = cst2[:, 2:3]
        a_col = cst2[:, 3:4]
        b3_col = cst2[:, 4:5]
        b2_col = cst2[:, 5:6]

        # PSUM->SBUF drains and element ops with an engine choice.
        def drain(dst_ap, src_ap, eng):
            if eng == "act":
                nc.scalar.activation(dst_ap, src_ap, AF.Copy)
            else:
                nc.vector.tensor_copy(dst_ap, src_ap)

        # Warmup: the PE runs at 0.65/1.2 GHz until ~3us of continuous
        # execution, and the first Activation pays a 1.3us table load.
        # Burn both on dummy reads of cstb while the first x pieces are
        # still in flight, so real work starts at full speed.
        if CFG["warmup_act"]:
            wact = consts.tile([128, 8], F32, tag="wact")
            nc.scalar.activation(wact[:], cstb[:, 0:8], AF.Exp)
        if CFG["warmup_mm"]:
            with tc.tile_pool(name="pwarm", bufs=1, space="PSUM") as pwarm:
                wps = pwarm.tile([64, 512], F32, tag="w")
                for _ in range(CFG["warmup_mm"]):
                    nc.tensor.matmul(wps[:], cstb[:, 0:64], cstb[:, 0:512],
                                     start=True, stop=True)

        import contextlib
        loop_cm = tc.For_i(0, loop_reps, 1) if loop_reps else \
            contextlib.nullcontext()
        with loop_cm:
            _body(nc, tc, locals(), variant)
    nc.compile()
    return nc


def _body(nc, tc, env, variant):
    px = env["px"]
    psT_ps, psT_sb = env["psT_ps"], env["psT_sb"]
    ps_ps, ps_sb, po_ps = env["ps_ps"], env["ps_sb"], env["po_ps"]
    pep, pout = env["pep"], env["pout"]
    xs_d, out_d, cst2_d = env["xs_d"], env["out_d"], env["cst2_d"]
    kern, identr, a2t = env["kern"], env["identr"], env["a2t"]
    inv_a, bias_exp, rbias = env["inv_a"], env["bias_exp"], env["rbias"]
    a_col, b3_col, b2_col = env["a_col"], env["b3_col"], env["b2_col"]
    cst2 = env["cst2"]
    drain = env["drain"]

    def ts(eng, *a, **k):
        (nc.vector if eng == "dve" else nc.gpsimd).tensor_scalar(*a, **k)

    def tt_add(eng, out, x, y):
        if eng == "dve":
            nc.vector.tensor_add(out, x, y)
        else:
            nc.gpsimd.tensor_add(out, x, y)

    def epi(zps, outsb, ob, pb):
        # epilogue: per-partition constants (d = partition % 64)
        if variant == "ln":
            # q = a*exp(y), y = z+bias. out = a*elu(y) + b2
            #   = max(a*y + b2, min(q, a) + (b2-a))  [y <= e^y - 1]
            q = pep.tile([128, 512], BF16, tag="q")
            t1 = pep.tile([128, 512], BF16, tag="t")
            nc.scalar.activation(q[:], zps[:], AF.Exp,
                                 bias=bias_exp, scale=inv_a)
            ts(_pick(CFG["t_engine"], pb), t1[:], q[:], a_col, b3_col,
               OP.min, OP.add)
            stt_eng = _pick(CFG["add_engine"], pb)
            (nc.vector if stt_eng == "dve" else
             nc.gpsimd).scalar_tensor_tensor(
                outsb[:, ob:ob + 512], zps[:], rbias, t1[:],
                OP.add, OP.max)
        else:
            q = pep.tile([128, 512], BF16, tag="q")
            nc.scalar.activation(q[:], zps[:], AF.Exp,
                                 bias=bias_exp, scale=inv_a)
            r = pep.tile([128, 512], BF16, tag="r")
            if _pick(CFG["r_engine"], pb) == "act":
                nc.scalar.activation(r[:], zps[:], AF.Relu, bias=rbias)
            else:
                ts(_pick(CFG["r_engine"], pb), r[:], zps[:], rbias, 0.0,
                   OP.add, OP.max)
            # safe for a<=0: q=exp(z+bias), r=relu(z+bias);
            # elu = r + min(q-1, 0); out = a*elu + b2
            t1 = pep.tile([128, 512], BF16, tag="t")
            ts(_pick(CFG["t_engine"], pb), t1[:], q[:], 1.0, 0.0,
               OP.subtract, OP.min)
            s1 = pep.tile([128, 512], BF16, tag="s")
            tt_add(_pick(CFG["add_engine"], pb), s1[:], t1[:], r[:])
            ts("dve", outsb[:, ob:ob + 512], s1[:], a_col, b2_col,
               OP.mult, OP.add)

    xsT_v = xs_d.rearrange("(j p) n -> p j n", p=128)
    LL = CFG["load_lbs"]
    SL = CFG["store_lbs"]
    for lb in range(NLB):
        if lb % LL == 0:
            xsb = px.tile([128, 4 * LL * LB_ROWS], BF16, tag="x")
            xsb_v = xsb[:].rearrange("p (j n) -> p j n", j=4)
            # Split the first/last loads so compute starts early
            if lb == 0:
                pieces = list(CFG["head_pieces"])
                rest = LL * LB_ROWS - sum(pieces)
                pieces += [rest] if rest else []
            elif lb == NLB - LL and CFG["tailsplit"]:
                pieces = [1024] * (LL * LB_ROWS // 1024)
            else:
                pieces = [LL * LB_ROWS]
            n0 = 0
            for pi, pn in enumerate(pieces):
                nc.sync.dma_start(
                    xsb_v[:, :, n0:n0 + pn],
                    xsT_v[:, :, lb * LB_ROWS + n0:lb * LB_ROWS + n0 + pn],
                )
                n0 += pn
                if lb == 0 and pi == 0:
                    # tiny f32 constant columns; issued after the first x
                    # piece so they don't delay the pipeline start
                    nc.sync.dma_start(cst2[:], cst2_d)
        nw0 = (lb % LL) * LB_ROWS
        if lb % SL == 0:
            outsb = pout.tile([128, SL * 2 * 512], BF16, tag="out")
        for pb in range(2):
            if CFG["direct_s1"]:
                # stage 1 direct: support[n, d] = xT_chunk.T @ kern per
                # 128-row group (x stationary, kern moving) — no PE
                # transposes, no supportT drains. Groups (2m, 2m+1) land
                # side by side as stage-2 chunk m.
                ssb = ps_sb.tile([128, 512], BF16, tag="ss")
                zps = po_ps.tile([128, 512], F32, tag="op")
                sp = psT_ps.tile([128, 512], F32, tag="sTp")
                dw = CFG["d1_drainw"]
                for m in range(4):
                    for g2 in range(2):
                        g8 = 2 * m + g2
                        r0 = nw0 + 1024 * pb + 128 * g8
                        oc = 128 * m + 64 * g2
                        for j in range(4):
                            nc.tensor.matmul(
                                sp[:, oc:oc + 64],
                                xsb_v[:, j, r0:r0 + 128],
                                kern[:, 64 * j:64 * (j + 1)],
                                start=(j == 0),
                                stop=(j == 3),
                            )
                    c1 = 128 * (m + 1)
                    if c1 % dw == 0:
                        c0 = c1 - dw
                        drain(ssb[:, c0:c1], sp[:, c0:c1],
                              _pick(CFG["sT_engine"], 2 * pb + m))
                        for mm in range(c0 // 128, c1 // 128):
                            nc.tensor.matmul(
                                zps[:, 128 * mm:128 * (mm + 1)],
                                ssb[:, 128 * mm:128 * (mm + 1)],
                                a2t, start=True, stop=True,
                            )
                epi(zps, outsb, 1024 * (lb % SL) + 512 * pb, pb)
                continue
            # stage 1: supportT [d, n]; one [128,512] tile or two [64,512]
            if CFG["sT_split"]:
                sT_views = []
                for gl in range(2):
                    g = 2 * pb + gl
                    sTps = psT_ps.tile([64, 512], F32, tag="sTp")
                    for j in range(4):
                        nc.tensor.matmul(
                            sTps[:],
                            kern[:, 64 * j:64 * (j + 1)],
                            xsb_v[:, j, nw0 + 512 * g:nw0 + 512 * (g + 1)],
                            start=(j == 0),
                            stop=(j == 3),
                        )
                    sTsb = psT_sb.tile([64, 512], BF16, tag="sTs")
                    drain(sTsb[:], sTps[:],
                          _pick(CFG["sT_engine"], 2 * pb + gl))
                    sT_views.append((sTsb, 0))
            else:
                sTps = psT_ps.tile([128, 512], F32, tag="sTp")
                for gl in range(2):
                    g = 2 * pb + gl
                    for j in range(4):
                        nc.tensor.matmul(
                            sTps[64 * gl:64 * (gl + 1), :],
                            kern[:, 64 * j:64 * (j + 1)],
                            xsb_v[:, j, nw0 + 512 * g:nw0 + 512 * (g + 1)],
                            start=(j == 0),
                            stop=(j == 3),
                        )
                sTsb = psT_sb.tile([128, 512], BF16, tag="sTs")
                drain(sTsb[:], sTps[:], _pick(CFG["sT_engine"], pb))
                sT_views = [(sTsb, 0), (sTsb, 64)]
            # transpose supportT -> support chunks [n, (gl,t,d)], then
            # drain + stage 2 per half so halves pipeline
            ssb = ps_sb.tile([128, 512], BF16, tag="ss")
            zps = po_ps.tile([128, 512], F32, tag="op")
            if not CFG["ssb_split"]:
                sps = ps_ps.tile([128, 512], BF16, tag="sp")
            for gl in range(2):
                src, p0 = sT_views[gl]
                ident = identr[p0:p0 + 64, p0:p0 + 64] if p0 else \
                    identr[:64, :64]
                if CFG["ssb_split"]:
                    # per-gl [128,256] PSUM tile: half the bank footprint
                    sps_g = ps_ps.tile([128, 256], BF16, tag="sp")
                    for t in range(4):
                        nc.tensor.transpose(
                            sps_g[:, 64 * t:64 * (t + 1)],
                            src[p0:p0 + 64, 128 * t:128 * (t + 1)],
                            ident,
                        )
                    h0 = 256 * gl
                    drain(ssb[:, h0:h0 + 256], sps_g[:],
                          _pick(CFG["ssb_engine"], 2 * pb + gl))
                    for m in (2 * gl, 2 * gl + 1):
                        nc.tensor.matmul(
                            zps[:, 128 * m:128 * (m + 1)],
                            ssb[:, 128 * m:128 * (m + 1)],
                            a2t, start=True, stop=True,
                        )
                else:
                    for t in range(4):
                        nc.tensor.transpose(
                            sps[:, 256 * gl + 64 * t:256 * gl + 64 * (t + 1)],
                            src[p0:p0 + 64, 128 * t:128 * (t + 1)],
                            ident,
                        )
            if not CFG["ssb_split"]:
                drain(ssb[:], sps[:], _pick(CFG["ssb_engine"], pb))
                for m in range(4):
                    nc.tensor.matmul(
                        zps[:, 128 * m:128 * (m + 1)],
                        ssb[:, 128 * m:128 * (m + 1)],
                        a2t, start=True, stop=True,
                    )
            epi(zps, outsb, 1024 * (lb % SL) + 512 * pb, pb)
        if lb % SL == SL - 1:
            # out DRAM is partition-major; host un-permutes
            c0 = (lb - SL + 1) * 2 * 512
            if lb == NLB - 1 and CFG["split_last_store"]:
                for h in range(SL):
                    nc.sync.dma_start(
                        out_d[:, c0 + h * 1024:c0 + (h + 1) * 1024],
                        outsb[:, h * 1024:(h + 1) * 1024],
                    )
            else:
                nc.sync.dma_start(
                    out_d[:, c0:c0 + SL * 1024], outsb[:],
                )


def get_nc(variant="ln"):
    if variant not in _NC_CACHE:
        _NC_CACHE[variant] = _build_nc(variant=variant)
    return _NC_CACHE[variant]


def host_prep(inputs):
    adj = np.asarray(inputs["adj_weight"], np.float32)
    kern = np.ascontiguousarray(np.asarray(inputs["kernel"], np.float32))
    bias = np.asarray(inputs["bias"], np.float32)
    gamma = np.asarray(inputs["gamma"], np.float32)
    beta = np.asarray(inputs["beta"], np.float32)
    mm = np.asarray(inputs["moving_mean"], np.float32)
    mv = np.asarray(inputs["moving_var"], np.float32)

    deg = np.maximum(np.abs(adj).sum(axis=1, keepdims=True), 1e-8)
    dis = deg ** -0.5
    adj_hat = adj * dis * dis.T + np.eye(C, dtype=np.float32)
    a2t = np.zeros((128, 128), np.float32)
    a2t[:64, :64] = adj_hat.T
    a2t[64:, 64:] = adj_hat.T

    a = (gamma / np.sqrt(mv + BN_EPS)).astype(np.float32)
    b2 = (beta - mm * a).astype(np.float32)
    variant = "ln" if np.all(a > 0) else "safe"

    # kern laid out [128, j, d]: kern_sb[p, j, d] = kernel[128 j + p, d],
    # with the BN scale folded in on the ln path
    kern_f = kern * a[None, :] if variant == "ln" else kern
    kern_t = kern_f.reshape(4, 128, D).transpose(1, 0, 2).reshape(128, 4 * D)

    cstb = np.zeros((128, 512), np.float32)
    cstb[:, 0:128] = np.eye(128, dtype=np.float32)
    cstb[:, 128:384] = kern_t
    cstb[:, 384:512] = a2t
    cstb = to_bf16(cstb)

    # per-partition constant columns: d = partition % 64
    dd = np.arange(128) % 64
    cst2 = np.zeros((128, 8), np.float32)
    if variant == "ln":
        cst2[:, 0] = (1.0 / a)[dd]
        cst2[:, 1] = (bias + np.log(a))[dd]
        cst2[:, 2] = (a * bias + b2)[dd]
    else:
        cst2[:, 0] = 1.0
        cst2[:, 1] = bias[dd]
        cst2[:, 2] = bias[dd]
    cst2[:, 3] = a[dd]
    cst2[:, 4] = (b2 - a)[dd]
    cst2[:, 5] = b2[dd]

    x = np.asarray(inputs["x"], np.float32)
    shards = x.reshape(NCORES, R, Fdim)
    import ml_dtypes
    in_maps = [
        {
            "xs": np.ascontiguousarray(to_bf16(shards[i]).T)
                  .view(ml_dtypes.bfloat16),
            "cstb": cstb.view(ml_dtypes.bfloat16),
            "cst2": cst2,
        }
        for i in range(NCORES)
    ]
    return in_maps, variant


def run(inputs, trace=False, **kw):
    in_maps, variant = host_prep(inputs)
    nc = get_nc(variant)
    try:
        res = bass_utils.run_bass_kernel_spmd(
            nc, in_maps, core_ids=list(range(NCORES)), trace=trace, **kw
        )
    except Exception:
        # transient NRT_EXEC_UNIT_UNRECOVERABLE has been observed right
        # after a previous process's teardown; one retry clears it
        import time as _time
        _time.sleep(5.0)
        res = bass_utils.run_bass_kernel_spmd(
            nc, in_maps, core_ids=list(range(NCORES)), trace=trace, **kw
        )
    shards = []
    for i in range(NCORES):
        raw = np.asarray(res.results[i]["out"]).astype(np.float32)
        # raw[p, C]: C = pbg*512 + 128*(2*gl+tq) + 64*h + c,
        # p = 64*ph + d; n = pbg*1024 + gl*512 + (2*tq+ph)*128 + 64*h + c
        shards.append(
            raw.reshape(2, 64, 16, 2, 2, 2, 64)
               .transpose(2, 3, 4, 0, 5, 6, 1)
               .reshape(R, D)
        )
    out = np.concatenate(shards, axis=0).reshape(B_FULL, C, D)
    return out, res


def kernel(**inputs) -> np.ndarray:
    out, _ = run(inputs)
    return out



# revision 56
# speedup vs baseline: 1.6961x; 1.6961x over previous
"""GCN message-passing kernel (nn_CARM_90185723281482) for 8 Trainium2 cores.

Computes, for x [2048, 64, 512], adj_weight [64, 64], kernel [512, 64]:
    adj_hat = D^-1/2 A D^-1/2 + I          (degree from row sums of |A|)
    out = BN(elu(adj_hat @ (x @ kernel) + bias))        -> [2048, 64, 64]

Sharding: data-parallel over the batch axis, 256 batches per core.
Per-core dataflow (rows n = (batch, channel) flattened, R = 16384 rows):
  - x ships bf16, host-pre-transposed: xs[p, j, n] = x2d[n, 128 j + p]
  - load block LB = 2048 rows (4 MiB per 2-LB DMA); first LB split small so
    the pipeline starts early
  - stage 1: supportT[d, n] += kern_j.T @ xT_j into one [128, 512] PSUM tile
    (both 512-row halves of a pb stacked on partitions), single drain
  - PE-transpose supportT back to support chunks ssb [n, (gl,t,d)]
  - stage 2 TRANSPOSED: zT[f, n] = ssb_chunk.T @ a2t per 128-wide f-chunk,
    so d = partition % 64 — all BN/bias constants become per-partition
    scalars riding the ACT bias/scale ports and TensorScalarPtr operands
  - 3-op epilogue (a = gamma*rsqrt(var+eps) folded into the stage-1 kernel
    when a > 0, so z_a = a*z comes off the PE; y = z + bias):
        q = exp(inv_a*z_a + bias + ln a)  = a*exp(y)           [ACT]
        t = min(q, a) + (b2 - a)                               [DVE ts]
        out = max(z_a + a*bias + b2, t)                        [DVE stt]
    which equals a*elu(y) + b2 on both branches (y <= e^y - 1 makes the
    max select the relu branch exactly when y >= 0).
  - output stored transposed [f, n]; host un-permutes.

Tuned against the TimelineSim cost model: 69428 ns (baseline) -> 64106 ns.
HW-verified rel err 3.7e-3.
"""

import sys

import numpy as np

sys.path.insert(0, "/opt/trn_rl_repo")

import concourse.bass as bass  # noqa: E402
from concourse import bacc, bass_utils, mybir, tile  # noqa: E402

F32 = mybir.dt.float32
BF16 = mybir.dt.bfloat16
FP8 = mybir.dt.float8e3  # e3m4: 4 mantissa bits, max 15.5
AF = mybir.ActivationFunctionType
OP = mybir.AluOpType

NCORES = 8
B_FULL, C, Fdim, D = 2048, 64, 512, 64
R = (B_FULL // NCORES) * C  # 16384 rows per core
LB_ROWS = 2048              # rows per load block
NLB = R // LB_ROWS          # 8 load blocks
BN_EPS = 1e-3

_NC_CACHE = {}

# Scheduling/balance knobs (tuned against the TimelineSim cost model)
CFG = {
    "deferred_stores": 1,  # all x loads first, all out stores after (SBUF-resident)
    "xdt": "f8e3",         # x wire/SBUF dtype: "bf16" | "f8e3" (halves DMA)
    "px": 8,
    "psT_sb": 8,
    "ps_sb": 5,
    "pep": 6,
    "psT_ps": 3,
    "ps_ps": 2,
    "po_ps": 5,
    "sT_split": 1,         # stage-1 PSUM: 0 = one [128,512], 1 = two [64,512]
    "ssb_split": 0,        # support drain: 0 = whole, 1 = per 256-col half
    "sT_engine": ["act"],          # stage-1 drain engine (per global pb)
    "ssb_engine": ["dve", "act"],  # support drain engine (legacy path)
    "r_engine": "act",     # relu (safe variant): "act" | "dve" | "pool"
    "t_engine": "dve",     # min/add tensor_scalar
    # final scalar_tensor_tensor reads PSUM, so DVE only (GPSIMD cannot
    # access PSUM on hardware -- the cost model wrongly allows it)
    "add_engine": ["dve"],
    "epi_v2": 0,           # split linear branch (slower here; see notes)
    "r_engine2": ["gps"],  # v2 linear-branch engine
    "store_lbs": 4,
    "tailsplit": 1,
    "tailpiece": 512,      # last-LB load piece rows (deferred path)
    "tail256": 1,          # split the final load piece in two
    "fine2": 0,            # 3-window last pb races with tail256 on HW; keep halves
    "fine_lbs": 1,         # trailing LBs using fine drain/epilogue
    "interleave_pb": 1,    # coarse LBs: stage-1 of both pbs back-to-back
    "lb_rows": 2048,       # rows per load block (deferred path)
    "s2_depth": 1,         # LBs of lookahead before stage-2/epilogue
    "drain_split": 0,      # coarse drains: halves on ACT+DVE in parallel
    "fillers": (0, 0, 0),  # PE keep-alive matmuls (tile scheduler hoists
                           # dependency-free work early, so these are inert)
    "load_lbs": 1,
    "split_last_store": 1,
    "warmup_mm": 0,        # dummy matmuls (reading cstb) to ramp the PE
    "warmup_act": 0,       # dummy Exp to preload the ACT table early
    "head_pieces": (),
    "ep_split": 0,         # epilogue per 256-col half
    "direct_s1": 1,        # stage-1 with x stationary: no transposes/drains
    "d1_drainw": 512,      # direct stage-1 drain width (128|256|512)
}


def _pick(v, pb):
    """Engine knob: either a name or a [pb0, pb1] alternation list."""
    return v[pb % len(v)] if isinstance(v, (list, tuple)) else v


def to_bf16(a):
    """fp32 -> bf16 (RNE), returned as a uint16 array (raw bf16 bits)."""
    u = np.ascontiguousarray(a, np.float32).view(np.uint32).astype(np.uint64)
    r = (u + 0x7FFF + ((u >> 16) & 1)) >> 16
    return r.astype(np.uint16)


def _build_nc(loop_reps=None, variant="ln"):
    nc = bacc.Bacc(
        "TRN2", target_bir_lowering=False, debug=False, num_devices=NCORES
    )
    cstb_w = 384 if CFG["direct_s1"] else 512
    xdt = FP8 if CFG["xdt"] == "f8e3" else BF16
    xs_d = nc.dram_tensor("xs", [Fdim, R], xdt, kind="ExternalInput").ap()
    cstb_d = nc.dram_tensor("cstb", [128, cstb_w], BF16,
                            kind="ExternalInput").ap()
    cst2_d = nc.dram_tensor("cst2", [128, 8], F32, kind="ExternalInput").ap()
    out_d = nc.dram_tensor("out", [128, (R // 128) * D], BF16,
                           kind="ExternalOutput").ap()

    with tile.TileContext(nc) as tc, \
         tc.tile_pool(name="consts", bufs=1) as consts, \
         tc.tile_pool(name="px", bufs=CFG["px"]) as px, \
         tc.tile_pool(name="psT_ps", bufs=CFG["psT_ps"], space="PSUM") as psT_ps, \
         tc.tile_pool(name="psT_sb", bufs=CFG["psT_sb"]) as psT_sb, \
         tc.tile_pool(name="ps_ps", bufs=CFG["ps_ps"], space="PSUM") as ps_ps, \
         tc.tile_pool(name="ps_sb", bufs=CFG["ps_sb"]) as ps_sb, \
         tc.tile_pool(name="po_ps", bufs=CFG["po_ps"], space="PSUM") as po_ps, \
         tc.tile_pool(name="pep", bufs=CFG["pep"]) as pep, \
         tc.tile_pool(name="pwarm", bufs=1, space="PSUM") as pwarm, \
         tc.tile_pool(name="pout", bufs=2) as pout:

        cstb = consts.tile([128, cstb_w], BF16, tag="cstb")
        if not CFG["deferred_stores"]:
            nc.sync.dma_start(cstb[:], cstb_d)
        cst2 = consts.tile([128, 8], F32, tag="cst2")
        if CFG["direct_s1"]:
            identr = None
            kern = cstb[:, 0:256]
            a2t = cstb[:, 256:384]
        else:
            identr = cstb[:, 0:128]
            kern = cstb[:, 128:384]
            a2t = cstb[:, 384:512]
        inv_a = cst2[:, 0:1]
        bias_exp = cst2[:, 1:2]
        rbias = cst2[:, 2:3]
        a_col = cst2[:, 3:4]
        b3_col = cst2[:, 4:5]
        b2_col = cst2[:, 5:6]

        # PSUM->SBUF drains and element ops with an engine choice.
        def drain(dst_ap, src_ap, eng):
            if eng == "act":
                nc.scalar.activation(dst_ap, src_ap, AF.Copy)
            else:
                nc.vector.tensor_copy(dst_ap, src_ap)

        # Warmup: the PE runs at 0.65/1.2 GHz until ~3us of continuous
        # execution, and the first Activation pays a 1.3us table load.
        # Burn both on dummy reads of cstb while the first x pieces are
        # still in flight, so real work starts at full speed.
        if CFG["warmup_act"]:
            wact = consts.tile([128, 8], F32, tag="wact")
            nc.scalar.activation(wact[:], cstb[:, 0:8], AF.Exp)
        if CFG["warmup_mm"]:
            with tc.tile_pool(name="pwarm", bufs=1, space="PSUM") as pwarm:
                wps = pwarm.tile([64, 512], F32, tag="w")
                for _ in range(CFG["warmup_mm"]):
                    nc.tensor.matmul(wps[:], cstb[:, 0:64], cstb[:, 0:512],
                                     start=True, stop=True)

        import contextlib
        loop_cm = tc.For_i(0, loop_reps, 1) if loop_reps else \
            contextlib.nullcontext()
        with loop_cm:
            _body(nc, tc, locals(), variant)
    nc.compile()
    return nc


def _body(nc, tc, env, variant):
    px = env["px"]
    psT_ps, psT_sb = env["psT_ps"], env["psT_sb"]
    ps_ps, ps_sb, po_ps = env["ps_ps"], env["ps_sb"], env["po_ps"]
    pep, pout = env["pep"], env["pout"]
    xs_d, out_d, cst2_d = env["xs_d"], env["out_d"], env["cst2_d"]
    kern, identr, a2t = env["kern"], env["identr"], env["a2t"]
    inv_a, bias_exp, rbias = env["inv_a"], env["bias_exp"], env["rbias"]
    a_col, b3_col, b2_col = env["a_col"], env["b3_col"], env["b2_col"]
    cst2 = env["cst2"]
    cstb, cstb_d = env["cstb"], env["cstb_d"]
    xdt = env["xdt"]
    drain = env["drain"]

    def ts(eng, *a, **k):
        (nc.vector if eng == "dve" else nc.gpsimd).tensor_scalar(*a, **k)

    def tt_add(eng, out, x, y):
        if eng == "dve":
            nc.vector.tensor_add(out, x, y)
        else:
            nc.gpsimd.tensor_add(out, x, y)

    def epi_range(zps, outsb, ob, pb, c0, cw):
        # ln epilogue on cols [c0, c0+cw): q = a*exp(y), y = z+bias.
        # out = a*elu(y) + b2 = max(a*y + b2, min(q, a) + (b2-a))
        q = pep.tile([128, cw], BF16, tag=f"q{c0}_{cw}")
        t1 = pep.tile([128, cw], BF16, tag=f"t{c0}_{cw}")
        nc.scalar.activation(q[:], zps[:, c0:c0 + cw], AF.Exp,
                             bias=bias_exp, scale=inv_a)
        if CFG["epi_v2"]:
            # Both PSUM readers (q, r) fire right after stage-2, so the
            # zps bank recycles ~1.2us sooner — without this, stage-2 of
            # pb k+po_ps stalls on the slow final op via the WAR ring.
            r = pep.tile([128, cw], BF16, tag=f"r{c0}_{cw}")
            r_eng = _pick(CFG["r_engine2"], pb)
            if r_eng == "act":
                nc.scalar.activation(r[:], zps[:, c0:c0 + cw], AF.Copy,
                                     bias=rbias)
            else:
                ts(r_eng, r[:], zps[:, c0:c0 + cw],
                   rbias, 0.0, OP.add, OP.bypass)
            ts(_pick(CFG["t_engine"], pb), t1[:], q[:], a_col, b3_col,
               OP.min, OP.add)
            f_eng = _pick(CFG["add_engine"], pb)
            (nc.vector if f_eng == "dve" else
             nc.gpsimd).scalar_tensor_tensor(
                outsb[:, ob + c0:ob + c0 + cw], r[:], 0.0,
                t1[:], OP.add, OP.max)
            return
        ts(_pick(CFG["t_engine"], pb), t1[:], q[:], a_col, b3_col,
           OP.min, OP.add)
        stt_eng = _pick(CFG["add_engine"], pb)
        (nc.vector if stt_eng == "dve" else
         nc.gpsimd).scalar_tensor_tensor(
            outsb[:, ob + c0:ob + c0 + cw], zps[:, c0:c0 + cw], rbias,
            t1[:], OP.add, OP.max)

    def epi(zps, outsb, ob, pb):
        # epilogue: per-partition constants (d = partition % 64)
        if variant == "ln":
            epi_range(zps, outsb, ob, pb, 0, 512)
        else:
            q = pep.tile([128, 512], BF16, tag="q")
            nc.scalar.activation(q[:], zps[:], AF.Exp,
                                 bias=bias_exp, scale=inv_a)
            r = pep.tile([128, 512], BF16, tag="r")
            if _pick(CFG["r_engine"], pb) == "act":
                nc.scalar.activation(r[:], zps[:], AF.Relu, bias=rbias)
            else:
                ts(_pick(CFG["r_engine"], pb), r[:], zps[:], rbias, 0.0,
                   OP.add, OP.max)
            # safe for a<=0: q=exp(z+bias), r=relu(z+bias);
            # elu = r + min(q-1, 0); out = a*elu + b2
            t1 = pep.tile([128, 512], BF16, tag="t")
            ts(_pick(CFG["t_engine"], pb), t1[:], q[:], 1.0, 0.0,
               OP.subtract, OP.min)
            s1 = pep.tile([128, 512], BF16, tag="s")
            tt_add(_pick(CFG["add_engine"], pb), s1[:], t1[:], r[:])
            ts("dve", outsb[:, ob:ob + 512], s1[:], a_col, b2_col,
               OP.mult, OP.add)

    xsT_v = xs_d.rearrange("(j p) n -> p j n", p=128)
    LL = CFG["load_lbs"]
    SL = CFG["store_lbs"]

    def direct_pb(xsb_v, nw0, pb, outsb, ob, fine=False, fill_mid=0,
                  fill=None, gkey=None):
        # stage 1 direct: support[n, d] = xT_chunk.T @ kern per 128-row
        # group (x stationary, kern moving); groups (2m, 2m+1) land side
        # by side as stage-2 chunk m.  fine=True pipelines drain/stage-2/
        # epilogue per 256-col half to shrink the end-of-kernel chain.
        if gkey is None:
            gkey = pb
        ssb = ps_sb.tile([128, 512], BF16, tag="ss")
        zps = po_ps.tile([128, 512], F32, tag="op")
        sp = psT_ps.tile([128, 512], F32, tag="sTp")
        dw = 256 if fine else CFG["d1_drainw"]

        def s1_group(m, tgt):
            for g2 in range(2):
                g8 = 2 * m + g2
                r0 = nw0 + 1024 * pb + 128 * g8
                oc = 128 * m + 64 * g2
                for j in range(4):
                    nc.tensor.matmul(
                        tgt[:, oc:oc + 64],
                        xsb_v[:, j, r0:r0 + 128],
                        kern[:, 64 * j:64 * (j + 1)],
                        start=(j == 0),
                        stop=(j == 3),
                    )

        def s2_chunks(c0, c1, z):
            for mm in range(c0 // 128, c1 // 128):
                nc.tensor.matmul(
                    z[:, 128 * mm:128 * (mm + 1)],
                    ssb[:, 128 * mm:128 * (mm + 1)],
                    a2t, start=True, stop=True,
                )

        if fine and variant == "ln":
            # Tail chain per window.  Each window gets its OWN PSUM banks:
            # the cost model serializes a PE write and an ACT/DVE read of
            # the same PSUM bank, so sharing banks between windows would
            # stall the next window's stage-1 behind this window's drain
            # and its stage-2 behind this window's exp.  fine==2 puts the
            # last 128 cols in their own window so the final load piece
            # feeds a minimal chain.
            wins = [(0, 2), (2, 3), (3, 4)] if fine == 2 else \
                [(0, 2), (2, 4)]
            for wi, (m0, m1) in enumerate(wins):
                tgt = sp if wi == 0 else psT_ps.tile([128, 512], F32,
                                                     tag="sTp")
                z = zps if wi == 0 else po_ps.tile([128, 512], F32,
                                                   tag="op")
                for m in range(m0, m1):
                    s1_group(m, tgt)
                c0, c1 = 128 * m0, 128 * m1
                drain(ssb[:, c0:c1], tgt[:, c0:c1],
                      _pick(CFG["sT_engine"], 2 * gkey + wi))
                s2_chunks(c0, c1, z)
                epi_range(z, outsb, ob, gkey, c0, c1 - c0)
            return

        for m in range(4):
            if m == 2 and fill_mid:
                fill(fill_mid)
            s1_group(m, sp)
            c1 = 128 * (m + 1)
            if c1 % dw == 0:
                c0 = c1 - dw
                drain(ssb[:, c0:c1], sp[:, c0:c1],
                      _pick(CFG["sT_engine"], 2 * gkey + m))
                s2_chunks(c0, c1, zps)
        epi(zps, outsb, ob, gkey)

    def direct_lb_s1(xsb_v, lb, npb, OW):
        # Stage-1 + drain for every pb of an LB, stage-1 back-to-back on
        # the PE so a pb's drain round-trip (PE -> ACT/DVE -> PE) hides
        # under the next pb's stage-1.  Separate PSUM banks per pb.
        # Stage-2/epilogue are emitted by the caller s2_depth LBs later,
        # so the drain never head-of-line-blocks a later LB's stage-1 on
        # the in-order PE queue.
        parts = []
        for pb in range(npb):
            gkey = npb * lb + pb
            ssb = ps_sb.tile([128, 512], BF16, tag="ss")
            sp = psT_ps.tile([128, 512], F32, tag="sTp")

            for m in range(4):
                for g2 in range(2):
                    g8 = 2 * m + g2
                    r0 = 1024 * pb + 128 * g8
                    oc = 128 * m + 64 * g2
                    for j in range(4):
                        nc.tensor.matmul(
                            sp[:, oc:oc + 64],
                            xsb_v[:, j, r0:r0 + 128],
                            kern[:, 64 * j:64 * (j + 1)],
                            start=(j == 0),
                            stop=(j == 3),
                        )
            if CFG["drain_split"]:
                e0 = _pick(CFG["sT_engine"], gkey)
                e1 = "dve" if e0 == "act" else "act"
                drain(ssb[:, 0:256], sp[:, 0:256], e0)
                drain(ssb[:, 256:512], sp[:, 256:512], e1)
            else:
                drain(ssb[:], sp[:], _pick(CFG["sT_engine"], gkey))
            parts.append((OW * lb + 512 * pb, gkey, ssb))
        return parts

    def direct_lb_s2(parts, outsb):
        for ob, gkey, ssb in parts:
            zps = po_ps.tile([128, 512], F32, tag="op")
            for mm in range(4):
                nc.tensor.matmul(
                    zps[:, 128 * mm:128 * (mm + 1)],
                    ssb[:, 128 * mm:128 * (mm + 1)],
                    a2t, start=True, stop=True,
                )
            epi(zps, outsb, ob, gkey)

    if CFG["deferred_stores"]:
        assert CFG["direct_s1"]
        LBR = CFG["lb_rows"]
        nlb = R // LBR
        npb = LBR // 1024
        OW = LBR // 2
        # Phase 1: issue every x load up front (no buffer reuse, so the
        # load stream never waits on compute).
        xsb_views = []
        for lb in range(nlb):
            xsb = px.tile([128, 4 * LBR], xdt, tag="x")
            xsb_v = xsb[:].rearrange("p (j n) -> p j n", j=4)
            xsb_views.append(xsb_v)
            if lb == 0 and CFG["head_pieces"]:
                pieces = list(CFG["head_pieces"])
                rest = LBR - sum(pieces)
                pieces += [rest] if rest else []
            elif lb == nlb - 1 and CFG["tailsplit"]:
                tp = CFG["tailpiece"]
                pieces = [tp] * (LBR // tp)
                if CFG["tail256"] and tp >= 512:
                    # split the final piece so the last-arriving data
                    # feeds only the m3 group of the last pb
                    pieces = pieces[:-1] + [tp // 2] * 2
            else:
                pieces = [LBR]
            n0 = 0
            for pi, pn in enumerate(pieces):
                nc.sync.dma_start(
                    xsb_v[:, :, n0:n0 + pn],
                    xsT_v[:, :, lb * LBR + n0:lb * LBR + n0 + pn],
                )
                n0 += pn
                if lb == 0 and pi == 0:
                    # constants ride behind the first x piece: their DGE
                    # overlaps its (long) transfer, so no stream gap
                    nc.sync.dma_start(cstb[:], cstb_d)
                    nc.sync.dma_start(cst2[:], cst2_d)
        # Phase 2: compute, all epilogues land in one persistent outsb.
        outsb = pout.tile([128, R // 2], BF16, tag="out")
        D = CFG["s2_depth"]
        pend = []
        for lb in range(nlb):
            xsb_v = xsb_views[lb]
            if CFG["interleave_pb"] and lb < nlb - CFG["fine_lbs"]:
                pend.append(direct_lb_s1(xsb_v, lb, npb, OW))
                if len(pend) > D:
                    direct_lb_s2(pend.pop(0), outsb)
                continue
            while pend:
                direct_lb_s2(pend.pop(0), outsb)
            last = lb == nlb - 1
            for pb in range(npb):
                gkey = npb * lb + pb
                fine = 2 if (last and pb == npb - 1 and CFG["fine2"]) else 1
                direct_pb(xsb_v, 0, pb, outsb, OW * lb + 512 * pb,
                          fine=fine, gkey=gkey)
        while pend:
            direct_lb_s2(pend.pop(0), outsb)
        # Phase 3: stores, queued behind every load on the SP queue.
        # Per-LB granularity; the final LB split in two so the last
        # transfer (and its post-DMA sem prop) is small and late-arriving
        # epilogues can't stall much ahead of it.
        for lb in range(nlb - 1):
            nc.sync.dma_start(
                out_d[:, OW * lb:OW * (lb + 1)],
                outsb[:, OW * lb:OW * (lb + 1)],
            )
        half = OW // 2
        for h in range(2):
            c0 = OW * (nlb - 1) + half * h
            nc.sync.dma_start(out_d[:, c0:c0 + half], outsb[:, c0:c0 + half])
        return

    for lb in range(NLB):
        if lb % LL == 0:
            xsb = px.tile([128, 4 * LL * LB_ROWS], xdt, tag="x")
            xsb_v = xsb[:].rearrange("p (j n) -> p j n", j=4)
            # Split the first/last loads so compute starts early
            if lb == 0:
                pieces = list(CFG["head_pieces"])
                rest = LL * LB_ROWS - sum(pieces)
                pieces += [rest] if rest else []
            elif lb == NLB - LL and CFG["tailsplit"]:
                pieces = [1024] * (LL * LB_ROWS // 1024)
            else:
                pieces = [LL * LB_ROWS]
            n0 = 0
            for pi, pn in enumerate(pieces):
                nc.sync.dma_start(
                    xsb_v[:, :, n0:n0 + pn],
                    xsT_v[:, :, lb * LB_ROWS + n0:lb * LB_ROWS + n0 + pn],
                )
                n0 += pn
                if lb == 0 and pi == 0:
                    # tiny f32 constant columns; issued after the first x
                    # piece so they don't delay the pipeline start
                    nc.sync.dma_start(cst2[:], cst2_d)
        nw0 = (lb % LL) * LB_ROWS
        if lb % SL == 0:
            outsb = pout.tile([128, SL * 2 * 512], BF16, tag="out")
        for pb in range(2):
            if CFG["direct_s1"]:
                # stage 1 direct: support[n, d] = xT_chunk.T @ kern per
                # 128-row group (x stationary, kern moving) — no PE
                # transposes, no supportT drains. Groups (2m, 2m+1) land
                # side by side as stage-2 chunk m.
                ssb = ps_sb.tile([128, 512], BF16, tag="ss")
                zps = po_ps.tile([128, 512], F32, tag="op")
                sp = psT_ps.tile([128, 512], F32, tag="sTp")
                dw = CFG["d1_drainw"]
                for m in range(4):
                    for g2 in range(2):
                        g8 = 2 * m + g2
                        r0 = nw0 + 1024 * pb + 128 * g8
                        oc = 128 * m + 64 * g2
                        for j in range(4):
                            nc.tensor.matmul(
                                sp[:, oc:oc + 64],
                                xsb_v[:, j, r0:r0 + 128],
                                kern[:, 64 * j:64 * (j + 1)],
                                start=(j == 0),
                                stop=(j == 3),
                            )
                    c1 = 128 * (m + 1)
                    if c1 % dw == 0:
                        c0 = c1 - dw
                        drain(ssb[:, c0:c1], sp[:, c0:c1],
                              _pick(CFG["sT_engine"], 2 * pb + m))
                        for mm in range(c0 // 128, c1 // 128):
                            nc.tensor.matmul(
                                zps[:, 128 * mm:128 * (mm + 1)],
                                ssb[:, 128 * mm:128 * (mm + 1)],
                                a2t, start=True, stop=True,
                            )
                epi(zps, outsb, 1024 * (lb % SL) + 512 * pb, pb)
                continue
            # stage 1: supportT [d, n]; one [128,512] tile or two [64,512]
            if CFG["sT_split"]:
                sT_views = []
                for gl in range(2):
                    g = 2 * pb + gl
                    sTps = psT_ps.tile([64, 512], F32, tag="sTp")
                    for j in range(4):
                        nc.tensor.matmul(
                            sTps[:],
                            kern[:, 64 * j:64 * (j + 1)],
                            xsb_v[:, j, nw0 + 512 * g:nw0 + 512 * (g + 1)],
                            start=(j == 0),
                            stop=(j == 3),
                        )
                    sTsb = psT_sb.tile([64, 512], BF16, tag="sTs")
                    drain(sTsb[:], sTps[:],
                          _pick(CFG["sT_engine"], 2 * pb + gl))
                    sT_views.append((sTsb, 0))
            else:
                sTps = psT_ps.tile([128, 512], F32, tag="sTp")
                for gl in range(2):
                    g = 2 * pb + gl
                    for j in range(4):
                        nc.tensor.matmul(
                            sTps[64 * gl:64 * (gl + 1), :],
                            kern[:, 64 * j:64 * (j + 1)],
                            xsb_v[:, j, nw0 + 512 * g:nw0 + 512 * (g + 1)],
                            start=(j == 0),
                            stop=(j == 3),
                        )
                sTsb = psT_sb.tile([128, 512], BF16, tag="sTs")
                drain(sTsb[:], sTps[:], _pick(CFG["sT_engine"], pb))
                sT_views = [(sTsb, 0), (sTsb, 64)]
            # transpose supportT -> support chunks [n, (gl,t,d)], then
            # drain + stage 2 per half so halves pipeline
            ssb = ps_sb.tile([128, 512], BF16, tag="ss")
            zps = po_ps.tile([128, 512], F32, tag="op")
            if not CFG["ssb_split"]:
                sps = ps_ps.tile([128, 512], BF16, tag="sp")
            for gl in range(2):
                src, p0 = sT_views[gl]
                ident = identr[p0:p0 + 64, p0:p0 + 64] if p0 else \
                    identr[:64, :64]
                if CFG["ssb_split"]:
                    # per-gl [128,256] PSUM tile: half the bank footprint
                    sps_g = ps_ps.tile([128, 256], BF16, tag="sp")
                    for t in range(4):
                        nc.tensor.transpose(
                            sps_g[:, 64 * t:64 * (t + 1)],
                            src[p0:p0 + 64, 128 * t:128 * (t + 1)],
                            ident,
                        )
                    h0 = 256 * gl
                    drain(ssb[:, h0:h0 + 256], sps_g[:],
                          _pick(CFG["ssb_engine"], 2 * pb + gl))
                    for m in (2 * gl, 2 * gl + 1):
                        nc.tensor.matmul(
                            zps[:, 128 * m:128 * (m + 1)],
                            ssb[:, 128 * m:128 * (m + 1)],
                            a2t, start=True, stop=True,
                        )
                else:
                    for t in range(4):
                        nc.tensor.transpose(
                            sps[:, 256 * gl + 64 * t:256 * gl + 64 * (t + 1)],
                            src[p0:p0 + 64, 128 * t:128 * (t + 1)],
                            ident,
                        )
            if not CFG["ssb_split"]:
                drain(ssb[:], sps[:], _pick(CFG["ssb_engine"], pb))
                for m in range(4):
                    nc.tensor.matmul(
                        zps[:, 128 * m:128 * (m + 1)],
                        ssb[:, 128 * m:128 * (m + 1)],
                        a2t, start=True, stop=True,
                    )
            epi(zps, outsb, 1024 * (lb % SL) + 512 * pb, pb)
        if lb % SL == SL - 1:
            # out DRAM is partition-major; host un-permutes
            c0 = (lb - SL + 1) * 2 * 512
            if lb == NLB - 1 and CFG["split_last_store"]:
                for h in range(SL):
                    nc.sync.dma_start(
                        out_d[:, c0 + h * 1024:c0 + (h + 1) * 1024],
                        outsb[:, h * 1024:(h + 1) * 1024],
                    )
            else:
                nc.sync.dma_start(
                    out_d[:, c0:c0 + SL * 1024], outsb[:],
                )


def get_nc(variant="ln"):
    if variant not in _NC_CACHE:
        _NC_CACHE[variant] = _build_nc(variant=variant)
    return _NC_CACHE[variant]


def host_prep(inputs):
    adj = np.asarray(inputs["adj_weight"], np.float32)
    kern = np.ascontiguousarray(np.asarray(inputs["kernel"], np.float32))
    bias = np.asarray(inputs["bias"], np.float32)
    gamma = np.asarray(inputs["gamma"], np.float32)
    beta = np.asarray(inputs["beta"], np.float32)
    mm = np.asarray(inputs["moving_mean"], np.float32)
    mv = np.asarray(inputs["moving_var"], np.float32)

    deg = np.maximum(np.abs(adj).sum(axis=1, keepdims=True), 1e-8)
    dis = deg ** -0.5
    adj_hat = adj * dis * dis.T + np.eye(C, dtype=np.float32)
    a2t = np.zeros((128, 128), np.float32)
    a2t[:64, :64] = adj_hat.T
    a2t[64:, 64:] = adj_hat.T

    a = (gamma / np.sqrt(mv + BN_EPS)).astype(np.float32)
    b2 = (beta - mm * a).astype(np.float32)
    variant = "ln" if np.all(a > 0) else "safe"

    # kern laid out [128, j, d]: kern_sb[p, j, d] = kernel[128 j + p, d],
    # with the BN scale folded in on the ln path
    kern_f = kern * a[None, :] if variant == "ln" else kern
    kern_t = kern_f.reshape(4, 128, D).transpose(1, 0, 2).reshape(128, 4 * D)

    if CFG["direct_s1"]:
        cstb = np.zeros((128, 384), np.float32)
        cstb[:, 0:256] = kern_t
        cstb[:, 256:384] = a2t
    else:
        cstb = np.zeros((128, 512), np.float32)
        cstb[:, 0:128] = np.eye(128, dtype=np.float32)
        cstb[:, 128:384] = kern_t
        cstb[:, 384:512] = a2t
    cstb = to_bf16(cstb)

    # per-partition constant columns: d = partition % 64
    dd = np.arange(128) % 64
    cst2 = np.zeros((128, 8), np.float32)
    if variant == "ln":
        cst2[:, 0] = (1.0 / a)[dd]
        cst2[:, 1] = (bias + np.log(a))[dd]
        cst2[:, 2] = (a * bias + b2)[dd]
    else:
        cst2[:, 0] = 1.0
        cst2[:, 1] = bias[dd]
        cst2[:, 2] = bias[dd]
    cst2[:, 3] = a[dd]
    cst2[:, 4] = (b2 - a)[dd]
    cst2[:, 5] = b2[dd]

    x = np.asarray(inputs["x"], np.float32)
    shards = x.reshape(NCORES, R, Fdim)
    import ml_dtypes
    if CFG["xdt"] == "f8e3":
        def xprep(s):
            return np.ascontiguousarray(s.T).astype(ml_dtypes.float8_e3m4)
    else:
        def xprep(s):
            return np.ascontiguousarray(to_bf16(s).T).view(ml_dtypes.bfloat16)
    in_maps = [
        {
            "xs": xprep(shards[i]),
            "cstb": cstb.view(ml_dtypes.bfloat16),
            "cst2": cst2,
        }
        for i in range(NCORES)
    ]
    return in_maps, variant


def run(inputs, trace=False, **kw):
    in_maps, variant = host_prep(inputs)
    nc = get_nc(variant)
    try:
        res = bass_utils.run_bass_kernel_spmd(
            nc, in_maps, core_ids=list(range(NCORES)), trace=trace, **kw
        )
    except Exception:
        # transient NRT_EXEC_UNIT_UNRECOVERABLE has been observed right
        # after a previous process's teardown; one retry clears it
        import time as _time
        _time.sleep(5.0)
        res = bass_utils.run_bass_kernel_spmd(
            nc, in_maps, core_ids=list(range(NCORES)), trace=trace, **kw
        )
    shards = []
    for i in range(NCORES):
        raw = np.asarray(res.results[i]["out"]).astype(np.float32)
        # raw[p, C]: C = pbg*512 + 128*(2*gl+tq) + 64*h + c,
        # p = 64*ph + d; n = pbg*1024 + gl*512 + (2*tq+ph)*128 + 64*h + c
        shards.append(
            raw.reshape(2, 64, 16, 2, 2, 2, 64)
               .transpose(2, 3, 4, 0, 5, 6, 1)
               .reshape(R, D)
        )
    out = np.concatenate(shards, axis=0).reshape(B_FULL, C, D)
    return out, res


def kernel(**inputs) -> np.ndarray:
    out, _ = run(inputs)
    return out



# revision 59
# speedup vs baseline: 1.7016x; 1.0032x over previous
"""GCN message-passing kernel (nn_CARM_90185723281482) for 8 Trainium2 cores.

Computes, for x [2048, 64, 512], adj_weight [64, 64], kernel [512, 64]:
    adj_hat = D^-1/2 A D^-1/2 + I          (degree from row sums of |A|)
    out = BN(elu(adj_hat @ (x @ kernel) + bias))        -> [2048, 64, 64]

Sharding: data-parallel over the batch axis, 256 batches per core.
Per-core dataflow (rows n = (batch, channel) flattened, R = 16384 rows):
  - x ships fp8 e3m4 (4 mantissa bits), host-pre-transposed:
    xs[p, j, n] = x2d[n, 128 j + p].  Halves the dominant DMA stream vs
    bf16; measured end-to-end rel err 1.41e-2 (gate 2e-2).  e4m3 fails
    the gate (2.7e-2); e3m4's extra mantissa bit is what makes fp8 x
    viable.  The PE takes mixed fp8-stationary x bf16-moving operands.
  - DMA schedule (the kernel is memory-bound): ALL x loads are issued up
    front into a fully SBUF-resident copy (px pool holds all 8 MiB; no
    buffer reuse, so the load stream never blocks on compute), and ALL
    output stores are queued behind them on the SP queue, waiting on
    their epilogues.  The DMA device then runs [loads | stores] nearly
    back-to-back; only the last LB's epilogue chain can expose a gap.
  - stage 1 per 1024-row pb: support[n, d] = x_chunk.T @ kern per
    128-row group (x stationary), accumulating 4 f-chunks into a
    [128, 512] PSUM tile; stage-1 of all pbs of an LB run back-to-back
    on the PE, with each pb's PSUM->SBUF drain (ACT) hidden under the
    next pb's stage-1, and stage-2 + epilogue of LB k emitted after
    stage-1 of LB k+1 (software pipelining across the in-order PE queue).
  - stage 2 TRANSPOSED: z[d, n] = ssb_chunk.T @ a2t per 128-wide chunk,
    so d = partition % 64 — all BN/bias constants become per-partition
    scalars riding the ACT bias/scale ports and TensorScalarPtr operands
  - 3-op epilogue (a = gamma*rsqrt(var+eps) folded into the stage-1
    kernel when a > 0, so z_a = a*z comes off the PE; y = z + bias):
        q = exp(inv_a*z_a + bias + ln a)  = a*exp(y)           [ACT]
        t = min(q, a) + (b2 - a)                               [DVE ts]
        out = max(z_a + a*bias + b2, t)                        [DVE stt]
    which equals a*elu(y) + b2 on both branches (y <= e^y - 1 makes the
    max select the relu branch exactly when y >= 0).  The stt reads PSUM
    so it must stay on DVE (GPSIMD cannot access PSUM on hardware, even
    though the cost model accepts it).
  - last LB: loads split into 512/256-row pieces and drain/stage-2/
    epilogue split per 256-col half with SEPARATE PSUM banks per half
    (the cost model serializes a PE write and an ACT/DVE read of the
    same bank), so the final store's wait clears early enough for its
    descriptor generation to overlap the previous store's transfer.
  - output stored transposed [f, n]; host un-permutes.

Tuned against the TimelineSim cost model:
  64106 ns (session baseline, bf16 x) -> 56330 ns (DMA schedule)
  -> 36925 ns (fp8 x + pipeline rebalance).  HW rel err 1.408e-2.
"""

import sys

import numpy as np

sys.path.insert(0, "/opt/trn_rl_repo")

import concourse.bass as bass  # noqa: E402
from concourse import bacc, bass_utils, mybir, tile  # noqa: E402

F32 = mybir.dt.float32
BF16 = mybir.dt.bfloat16
FP8 = mybir.dt.float8e3  # e3m4: 4 mantissa bits, max 15.5
AF = mybir.ActivationFunctionType
OP = mybir.AluOpType

NCORES = 8
B_FULL, C, Fdim, D = 2048, 64, 512, 64
R = (B_FULL // NCORES) * C  # 16384 rows per core
LB_ROWS = 2048              # rows per load block
NLB = R // LB_ROWS          # 8 load blocks
BN_EPS = 1e-3

_NC_CACHE = {}

# Scheduling/balance knobs (tuned against the TimelineSim cost model)
CFG = {
    "deferred_stores": 1,  # all x loads first, all out stores after (SBUF-resident)
    "xdt": "f8e3",         # x wire/SBUF dtype: "bf16" | "f8e3" (halves DMA)
    "px": 8,
    "psT_sb": 8,
    "ps_sb": 5,
    "pep": 6,
    "psT_ps": 3,
    "ps_ps": 2,
    "po_ps": 5,
    "sT_split": 1,         # stage-1 PSUM: 0 = one [128,512], 1 = two [64,512]
    "ssb_split": 0,        # support drain: 0 = whole, 1 = per 256-col half
    "sT_engine": ["act"],          # stage-1 drain engine (per global pb)
    "ssb_engine": ["dve", "act"],  # support drain engine (legacy path)
    "r_engine": "act",     # relu (safe variant): "act" | "dve" | "pool"
    "t_engine": ["dve"] * 14 + ["gps", "gps"],  # min/add tensor_scalar
    # final scalar_tensor_tensor reads PSUM, so DVE only (GPSIMD cannot
    # access PSUM on hardware -- the cost model wrongly allows it)
    "add_engine": ["dve"],
    "epi_v2": 0,           # split linear branch (slower here; see notes)
    "r_engine2": ["gps"],  # v2 linear-branch engine
    "store_lbs": 4,
    "tailsplit": 1,
    "tailpiece": 512,      # last-LB load piece rows (deferred path)
    "tail256": 1,          # split the final load piece in two
    "fine2": 0,            # 3-window last pb races with tail256 on HW; keep halves
    "fine_lbs": 1,         # trailing LBs using fine drain/epilogue
    "interleave_pb": 1,    # coarse LBs: stage-1 of both pbs back-to-back
    "lb_rows": 2048,       # rows per load block (deferred path)
    "s2_depth": 1,         # LBs of lookahead before stage-2/epilogue
    "drain_split": 0,      # coarse drains: halves on ACT+DVE in parallel
    "fillers": (0, 0, 0),  # PE keep-alive matmuls (tile scheduler hoists
                           # dependency-free work early, so these are inert)
    "load_lbs": 1,
    "split_last_store": 1,
    "split_store_lbs": 2,  # trailing LBs stored per-pb (half-width)
    "warmup_mm": 0,        # dummy matmuls (reading cstb) to ramp the PE
    "warmup_act": 0,       # dummy Exp to preload the ACT table early
    "head_pieces": (),
    "ep_split": 0,         # epilogue per 256-col half
    "direct_s1": 1,        # stage-1 with x stationary: no transposes/drains
    "d1_drainw": 512,      # direct stage-1 drain width (128|256|512)
}


def _pick(v, pb):
    """Engine knob: either a name or a [pb0, pb1] alternation list."""
    return v[pb % len(v)] if isinstance(v, (list, tuple)) else v


def to_bf16(a):
    """fp32 -> bf16 (RNE), returned as a uint16 array (raw bf16 bits)."""
    u = np.ascontiguousarray(a, np.float32).view(np.uint32).astype(np.uint64)
    r = (u + 0x7FFF + ((u >> 16) & 1)) >> 16
    return r.astype(np.uint16)


def _build_nc(loop_reps=None, variant="ln"):
    nc = bacc.Bacc(
        "TRN2", target_bir_lowering=False, debug=False, num_devices=NCORES
    )
    cstb_w = 384 if CFG["direct_s1"] else 512
    xdt = FP8 if CFG["xdt"] == "f8e3" else BF16
    xs_d = nc.dram_tensor("xs", [Fdim, R], xdt, kind="ExternalInput").ap()
    cstb_d = nc.dram_tensor("cstb", [128, cstb_w], BF16,
                            kind="ExternalInput").ap()
    cst2_d = nc.dram_tensor("cst2", [128, 8], F32, kind="ExternalInput").ap()
    out_d = nc.dram_tensor("out", [128, (R // 128) * D], BF16,
                           kind="ExternalOutput").ap()

    with tile.TileContext(nc) as tc, \
         tc.tile_pool(name="consts", bufs=1) as consts, \
         tc.tile_pool(name="px", bufs=CFG["px"]) as px, \
         tc.tile_pool(name="psT_ps", bufs=CFG["psT_ps"], space="PSUM") as psT_ps, \
         tc.tile_pool(name="psT_sb", bufs=CFG["psT_sb"]) as psT_sb, \
         tc.tile_pool(name="ps_ps", bufs=CFG["ps_ps"], space="PSUM") as ps_ps, \
         tc.tile_pool(name="ps_sb", bufs=CFG["ps_sb"]) as ps_sb, \
         tc.tile_pool(name="po_ps", bufs=CFG["po_ps"], space="PSUM") as po_ps, \
         tc.tile_pool(name="pep", bufs=CFG["pep"]) as pep, \
         tc.tile_pool(name="pwarm", bufs=1, space="PSUM") as pwarm, \
         tc.tile_pool(name="pout", bufs=2) as pout:

        cstb = consts.tile([128, cstb_w], BF16, tag="cstb")
        if not CFG["deferred_stores"]:
            nc.sync.dma_start(cstb[:], cstb_d)
        cst2 = consts.tile([128, 8], F32, tag="cst2")
        if CFG["direct_s1"]:
            identr = None
            kern = cstb[:, 0:256]
            a2t = cstb[:, 256:384]
        else:
            identr = cstb[:, 0:128]
            kern = cstb[:, 128:384]
            a2t = cstb[:, 384:512]
        inv_a = cst2[:, 0:1]
        bias_exp = cst2[:, 1:2]
        rbias = cst2[:, 2:3]
        a_col = cst2[:, 3:4]
        b3_col = cst2[:, 4:5]
        b2_col = cst2[:, 5:6]

        # PSUM->SBUF drains and element ops with an engine choice.
        def drain(dst_ap, src_ap, eng):
            if eng == "act":
                nc.scalar.activation(dst_ap, src_ap, AF.Copy)
            else:
                nc.vector.tensor_copy(dst_ap, src_ap)

        # Warmup: the PE runs at 0.65/1.2 GHz until ~3us of continuous
        # execution, and the first Activation pays a 1.3us table load.
        # Burn both on dummy reads of cstb while the first x pieces are
        # still in flight, so real work starts at full speed.
        if CFG["warmup_act"]:
            wact = consts.tile([128, 8], F32, tag="wact")
            nc.scalar.activation(wact[:], cstb[:, 0:8], AF.Exp)
        if CFG["warmup_mm"]:
            with tc.tile_pool(name="pwarm", bufs=1, space="PSUM") as pwarm:
                wps = pwarm.tile([64, 512], F32, tag="w")
                for _ in range(CFG["warmup_mm"]):
                    nc.tensor.matmul(wps[:], cstb[:, 0:64], cstb[:, 0:512],
                                     start=True, stop=True)

        import contextlib
        loop_cm = tc.For_i(0, loop_reps, 1) if loop_reps else \
            contextlib.nullcontext()
        with loop_cm:
            _body(nc, tc, locals(), variant)
    nc.compile()
    return nc


def _body(nc, tc, env, variant):
    px = env["px"]
    psT_ps, psT_sb = env["psT_ps"], env["psT_sb"]
    ps_ps, ps_sb, po_ps = env["ps_ps"], env["ps_sb"], env["po_ps"]
    pep, pout = env["pep"], env["pout"]
    xs_d, out_d, cst2_d = env["xs_d"], env["out_d"], env["cst2_d"]
    kern, identr, a2t = env["kern"], env["identr"], env["a2t"]
    inv_a, bias_exp, rbias = env["inv_a"], env["bias_exp"], env["rbias"]
    a_col, b3_col, b2_col = env["a_col"], env["b3_col"], env["b2_col"]
    cst2 = env["cst2"]
    cstb, cstb_d = env["cstb"], env["cstb_d"]
    xdt = env["xdt"]
    drain = env["drain"]

    def ts(eng, *a, **k):
        (nc.vector if eng == "dve" else nc.gpsimd).tensor_scalar(*a, **k)

    def tt_add(eng, out, x, y):
        if eng == "dve":
            nc.vector.tensor_add(out, x, y)
        else:
            nc.gpsimd.tensor_add(out, x, y)

    def epi_range(zps, outsb, ob, pb, c0, cw):
        # ln epilogue on cols [c0, c0+cw): q = a*exp(y), y = z+bias.
        # out = a*elu(y) + b2 = max(a*y + b2, min(q, a) + (b2-a))
        q = pep.tile([128, cw], BF16, tag=f"q{c0}_{cw}")
        t1 = pep.tile([128, cw], BF16, tag=f"t{c0}_{cw}")
        nc.scalar.activation(q[:], zps[:, c0:c0 + cw], AF.Exp,
                             bias=bias_exp, scale=inv_a)
        if CFG["epi_v2"]:
            # Both PSUM readers (q, r) fire right after stage-2, so the
            # zps bank recycles ~1.2us sooner — without this, stage-2 of
            # pb k+po_ps stalls on the slow final op via the WAR ring.
            r = pep.tile([128, cw], BF16, tag=f"r{c0}_{cw}")
            r_eng = _pick(CFG["r_engine2"], pb)
            if r_eng == "act":
                nc.scalar.activation(r[:], zps[:, c0:c0 + cw], AF.Copy,
                                     bias=rbias)
            else:
                ts(r_eng, r[:], zps[:, c0:c0 + cw],
                   rbias, 0.0, OP.add, OP.bypass)
            ts(_pick(CFG["t_engine"], pb), t1[:], q[:], a_col, b3_col,
               OP.min, OP.add)
            f_eng = _pick(CFG["add_engine"], pb)
            (nc.vector if f_eng == "dve" else
             nc.gpsimd).scalar_tensor_tensor(
                outsb[:, ob + c0:ob + c0 + cw], r[:], 0.0,
                t1[:], OP.add, OP.max)
            return
        ts(_pick(CFG["t_engine"], pb), t1[:], q[:], a_col, b3_col,
           OP.min, OP.add)
        stt_eng = _pick(CFG["add_engine"], pb)
        (nc.vector if stt_eng == "dve" else
         nc.gpsimd).scalar_tensor_tensor(
            outsb[:, ob + c0:ob + c0 + cw], zps[:, c0:c0 + cw], rbias,
            t1[:], OP.add, OP.max)

    def epi(zps, outsb, ob, pb):
        # epilogue: per-partition constants (d = partition % 64)
        if variant == "ln":
            epi_range(zps, outsb, ob, pb, 0, 512)
        else:
            q = pep.tile([128, 512], BF16, tag="q")
            nc.scalar.activation(q[:], zps[:], AF.Exp,
                                 bias=bias_exp, scale=inv_a)
            r = pep.tile([128, 512], BF16, tag="r")
            if _pick(CFG["r_engine"], pb) == "act":
                nc.scalar.activation(r[:], zps[:], AF.Relu, bias=rbias)
            else:
                ts(_pick(CFG["r_engine"], pb), r[:], zps[:], rbias, 0.0,
                   OP.add, OP.max)
            # safe for a<=0: q=exp(z+bias), r=relu(z+bias);
            # elu = r + min(q-1, 0); out = a*elu + b2
            t1 = pep.tile([128, 512], BF16, tag="t")
            ts(_pick(CFG["t_engine"], pb), t1[:], q[:], 1.0, 0.0,
               OP.subtract, OP.min)
            s1 = pep.tile([128, 512], BF16, tag="s")
            tt_add(_pick(CFG["add_engine"], pb), s1[:], t1[:], r[:])
            ts("dve", outsb[:, ob:ob + 512], s1[:], a_col, b2_col,
               OP.mult, OP.add)

    xsT_v = xs_d.rearrange("(j p) n -> p j n", p=128)
    LL = CFG["load_lbs"]
    SL = CFG["store_lbs"]

    def direct_pb(xsb_v, nw0, pb, outsb, ob, fine=False, fill_mid=0,
                  fill=None, gkey=None):
        # stage 1 direct: support[n, d] = xT_chunk.T @ kern per 128-row
        # group (x stationary, kern moving); groups (2m, 2m+1) land side
        # by side as stage-2 chunk m.  fine=True pipelines drain/stage-2/
        # epilogue per 256-col half to shrink the end-of-kernel chain.
        if gkey is None:
            gkey = pb
        ssb = ps_sb.tile([128, 512], BF16, tag="ss")
        zps = po_ps.tile([128, 512], F32, tag="op")
        sp = psT_ps.tile([128, 512], F32, tag="sTp")
        dw = 256 if fine else CFG["d1_drainw"]

        def s1_group(m, tgt):
            for g2 in range(2):
                g8 = 2 * m + g2
                r0 = nw0 + 1024 * pb + 128 * g8
                oc = 128 * m + 64 * g2
                for j in range(4):
                    nc.tensor.matmul(
                        tgt[:, oc:oc + 64],
                        xsb_v[:, j, r0:r0 + 128],
                        kern[:, 64 * j:64 * (j + 1)],
                        start=(j == 0),
                        stop=(j == 3),
                    )

        def s2_chunks(c0, c1, z):
            for mm in range(c0 // 128, c1 // 128):
                nc.tensor.matmul(
                    z[:, 128 * mm:128 * (mm + 1)],
                    ssb[:, 128 * mm:128 * (mm + 1)],
                    a2t, start=True, stop=True,
                )

        if fine and variant == "ln":
            # Tail chain per window.  Each window gets its OWN PSUM banks:
            # the cost model serializes a PE write and an ACT/DVE read of
            # the same PSUM bank, so sharing banks between windows would
            # stall the next window's stage-1 behind this window's drain
            # and its stage-2 behind this window's exp.  fine==2 puts the
            # last 128 cols in their own window so the final load piece
            # feeds a minimal chain.
            wins = [(0, 2), (2, 3), (3, 4)] if fine == 2 else \
                [(0, 2), (2, 4)]
            for wi, (m0, m1) in enumerate(wins):
                tgt = sp if wi == 0 else psT_ps.tile([128, 512], F32,
                                                     tag="sTp")
                z = zps if wi == 0 else po_ps.tile([128, 512], F32,
                                                   tag="op")
                for m in range(m0, m1):
                    s1_group(m, tgt)
                c0, c1 = 128 * m0, 128 * m1
                drain(ssb[:, c0:c1], tgt[:, c0:c1],
                      _pick(CFG["sT_engine"], 2 * gkey + wi))
                s2_chunks(c0, c1, z)
                epi_range(z, outsb, ob, gkey, c0, c1 - c0)
            return

        for m in range(4):
            if m == 2 and fill_mid:
                fill(fill_mid)
            s1_group(m, sp)
            c1 = 128 * (m + 1)
            if c1 % dw == 0:
                c0 = c1 - dw
                drain(ssb[:, c0:c1], sp[:, c0:c1],
                      _pick(CFG["sT_engine"], 2 * gkey + m))
                s2_chunks(c0, c1, zps)
        epi(zps, outsb, ob, gkey)

    def direct_lb_s1(xsb_v, lb, npb, OW):
        # Stage-1 + drain for every pb of an LB, stage-1 back-to-back on
        # the PE so a pb's drain round-trip (PE -> ACT/DVE -> PE) hides
        # under the next pb's stage-1.  Separate PSUM banks per pb.
        # Stage-2/epilogue are emitted by the caller s2_depth LBs later,
        # so the drain never head-of-line-blocks a later LB's stage-1 on
        # the in-order PE queue.
        parts = []
        for pb in range(npb):
            gkey = npb * lb + pb
            ssb = ps_sb.tile([128, 512], BF16, tag="ss")
            sp = psT_ps.tile([128, 512], F32, tag="sTp")

            for m in range(4):
                for g2 in range(2):
                    g8 = 2 * m + g2
                    r0 = 1024 * pb + 128 * g8
                    oc = 128 * m + 64 * g2
                    for j in range(4):
                        nc.tensor.matmul(
                            sp[:, oc:oc + 64],
                            xsb_v[:, j, r0:r0 + 128],
                            kern[:, 64 * j:64 * (j + 1)],
                            start=(j == 0),
                            stop=(j == 3),
                        )
            if CFG["drain_split"]:
                e0 = _pick(CFG["sT_engine"], gkey)
                e1 = "dve" if e0 == "act" else "act"
                drain(ssb[:, 0:256], sp[:, 0:256], e0)
                drain(ssb[:, 256:512], sp[:, 256:512], e1)
            else:
                drain(ssb[:], sp[:], _pick(CFG["sT_engine"], gkey))
            parts.append((OW * lb + 512 * pb, gkey, ssb))
        return parts

    def direct_lb_s2(parts, outsb):
        for ob, gkey, ssb in parts:
            zps = po_ps.tile([128, 512], F32, tag="op")
            for mm in range(4):
                nc.tensor.matmul(
                    zps[:, 128 * mm:128 * (mm + 1)],
                    ssb[:, 128 * mm:128 * (mm + 1)],
                    a2t, start=True, stop=True,
                )
            epi(zps, outsb, ob, gkey)

    if CFG["deferred_stores"]:
        assert CFG["direct_s1"]
        LBR = CFG["lb_rows"]
        nlb = R // LBR
        npb = LBR // 1024
        OW = LBR // 2
        # Phase 1: issue every x load up front (no buffer reuse, so the
        # load stream never waits on compute).
        xsb_views = []
        for lb in range(nlb):
            xsb = px.tile([128, 4 * LBR], xdt, tag="x")
            xsb_v = xsb[:].rearrange("p (j n) -> p j n", j=4)
            xsb_views.append(xsb_v)
            if lb == 0 and CFG["head_pieces"]:
                pieces = list(CFG["head_pieces"])
                rest = LBR - sum(pieces)
                pieces += [rest] if rest else []
            elif lb == nlb - 1 and CFG["tailsplit"]:
                tp = CFG["tailpiece"]
                pieces = [tp] * (LBR // tp)
                if CFG["tail256"] and tp >= 512:
                    # split the final piece so the last-arriving data
                    # feeds only the m3 group of the last pb
                    pieces = pieces[:-1] + [tp // 2] * 2
            else:
                pieces = [LBR]
            n0 = 0
            for pi, pn in enumerate(pieces):
                nc.sync.dma_start(
                    xsb_v[:, :, n0:n0 + pn],
                    xsT_v[:, :, lb * LBR + n0:lb * LBR + n0 + pn],
                )
                n0 += pn
                if lb == 0 and pi == 0:
                    # constants ride behind the first x piece: their DGE
                    # overlaps its (long) transfer, so no stream gap
                    nc.sync.dma_start(cstb[:], cstb_d)
                    nc.sync.dma_start(cst2[:], cst2_d)
        # Phase 2: compute, all epilogues land in one persistent outsb.
        outsb = pout.tile([128, R // 2], BF16, tag="out")
        D = CFG["s2_depth"]
        pend = []
        for lb in range(nlb):
            xsb_v = xsb_views[lb]
            if CFG["interleave_pb"] and lb < nlb - CFG["fine_lbs"]:
                pend.append(direct_lb_s1(xsb_v, lb, npb, OW))
                if len(pend) > D:
                    direct_lb_s2(pend.pop(0), outsb)
                continue
            while pend:
                direct_lb_s2(pend.pop(0), outsb)
            last = lb == nlb - 1
            for pb in range(npb):
                gkey = npb * lb + pb
                fine = 2 if (last and pb == npb - 1 and CFG["fine2"]) else 1
                direct_pb(xsb_v, 0, pb, outsb, OW * lb + 512 * pb,
                          fine=fine, gkey=gkey)
        while pend:
            direct_lb_s2(pend.pop(0), outsb)
        # Phase 3: stores, queued behind every load on the SP queue.
        # Per-LB granularity; the final LB split in two so the last
        # transfer (and its post-DMA sem prop) is small and late-arriving
        # epilogues can't stall much ahead of it.
        half = OW // 2
        for lb in range(nlb):
            if lb >= nlb - CFG["split_store_lbs"]:
                for h in range(2):
                    c0 = OW * lb + half * h
                    nc.sync.dma_start(out_d[:, c0:c0 + half],
                                      outsb[:, c0:c0 + half])
            else:
                nc.sync.dma_start(
                    out_d[:, OW * lb:OW * (lb + 1)],
                    outsb[:, OW * lb:OW * (lb + 1)],
                )
        return

    for lb in range(NLB):
        if lb % LL == 0:
            xsb = px.tile([128, 4 * LL * LB_ROWS], xdt, tag="x")
            xsb_v = xsb[:].rearrange("p (j n) -> p j n", j=4)
            # Split the first/last loads so compute starts early
            if lb == 0:
                pieces = list(CFG["head_pieces"])
                rest = LL * LB_ROWS - sum(pieces)
                pieces += [rest] if rest else []
            elif lb == NLB - LL and CFG["tailsplit"]:
                pieces = [1024] * (LL * LB_ROWS // 1024)
            else:
                pieces = [LL * LB_ROWS]
            n0 = 0
            for pi, pn in enumerate(pieces):
                nc.sync.dma_start(
                    xsb_v[:, :, n0:n0 + pn],
                    xsT_v[:, :, lb * LB_ROWS + n0:lb * LB_ROWS + n0 + pn],
                )
                n0 += pn
                if lb == 0 and pi == 0:
                    # tiny f32 constant columns; issued after the first x
                    # piece so they don't delay the pipeline start
                    nc.sync.dma_start(cst2[:], cst2_d)
        nw0 = (lb % LL) * LB_ROWS
        if lb % SL == 0:
            outsb = pout.tile([128, SL * 2 * 512], BF16, tag="out")
        for pb in range(2):
            if CFG["direct_s1"]:
                # stage 1 direct: support[n, d] = xT_chunk.T @ kern per
                # 128-row group (x stationary, kern moving) — no PE
                # transposes, no supportT drains. Groups (2m, 2m+1) land
                # side by side as stage-2 chunk m.
                ssb = ps_sb.tile([128, 512], BF16, tag="ss")
                zps = po_ps.tile([128, 512], F32, tag="op")
                sp = psT_ps.tile([128, 512], F32, tag="sTp")
                dw = CFG["d1_drainw"]
                for m in range(4):
                    for g2 in range(2):
                        g8 = 2 * m + g2
                        r0 = nw0 + 1024 * pb + 128 * g8
                        oc = 128 * m + 64 * g2
                        for j in range(4):
                            nc.tensor.matmul(
                                sp[:, oc:oc + 64],
                                xsb_v[:, j, r0:r0 + 128],
                                kern[:, 64 * j:64 * (j + 1)],
                                start=(j == 0),
                                stop=(j == 3),
                            )
                    c1 = 128 * (m + 1)
                    if c1 % dw == 0:
                        c0 = c1 - dw
                        drain(ssb[:, c0:c1], sp[:, c0:c1],
                              _pick(CFG["sT_engine"], 2 * pb + m))
                        for mm in range(c0 // 128, c1 // 128):
                            nc.tensor.matmul(
                                zps[:, 128 * mm:128 * (mm + 1)],
                                ssb[:, 128 * mm:128 * (mm + 1)],
                                a2t, start=True, stop=True,
                            )
                epi(zps, outsb, 1024 * (lb % SL) + 512 * pb, pb)
                continue
            # stage 1: supportT [d, n]; one [128,512] tile or two [64,512]
            if CFG["sT_split"]:
                sT_views = []
                for gl in range(2):
                    g = 2 * pb + gl
                    sTps = psT_ps.tile([64, 512], F32, tag="sTp")
                    for j in range(4):
                        nc.tensor.matmul(
                            sTps[:],
                            kern[:, 64 * j:64 * (j + 1)],
                            xsb_v[:, j, nw0 + 512 * g:nw0 + 512 * (g + 1)],
                            start=(j == 0),
                            stop=(j == 3),
                        )
                    sTsb = psT_sb.tile([64, 512], BF16, tag="sTs")
                    drain(sTsb[:], sTps[:],
                          _pick(CFG["sT_engine"], 2 * pb + gl))
                    sT_views.append((sTsb, 0))
            else:
                sTps = psT_ps.tile([128, 512], F32, tag="sTp")
                for gl in range(2):
                    g = 2 * pb + gl
                    for j in range(4):
                        nc.tensor.matmul(
                            sTps[64 * gl:64 * (gl + 1), :],
                            kern[:, 64 * j:64 * (j + 1)],
                            xsb_v[:, j, nw0 + 512 * g:nw0 + 512 * (g + 1)],
                            start=(j == 0),
                            stop=(j == 3),
                        )
                sTsb = psT_sb.tile([128, 512], BF16, tag="sTs")
                drain(sTsb[:], sTps[:], _pick(CFG["sT_engine"], pb))
                sT_views = [(sTsb, 0), (sTsb, 64)]
            # transpose supportT -> support chunks [n, (gl,t,d)], then
            # drain + stage 2 per half so halves pipeline
            ssb = ps_sb.tile([128, 512], BF16, tag="ss")
            zps = po_ps.tile([128, 512], F32, tag="op")
            if not CFG["ssb_split"]:
                sps = ps_ps.tile([128, 512], BF16, tag="sp")
            for gl in range(2):
                src, p0 = sT_views[gl]
                ident = identr[p0:p0 + 64, p0:p0 + 64] if p0 else \
                    identr[:64, :64]
                if CFG["ssb_split"]:
                    # per-gl [128,256] PSUM tile: half the bank footprint
                    sps_g = ps_ps.tile([128, 256], BF16, tag="sp")
                    for t in range(4):
                        nc.tensor.transpose(
                            sps_g[:, 64 * t:64 * (t + 1)],
                            src[p0:p0 + 64, 128 * t:128 * (t + 1)],
                            ident,
                        )
                    h0 = 256 * gl
                    drain(ssb[:, h0:h0 + 256], sps_g[:],
                          _pick(CFG["ssb_engine"], 2 * pb + gl))
                    for m in (2 * gl, 2 * gl + 1):
                        nc.tensor.matmul(
                            zps[:, 128 * m:128 * (m + 1)],
                            ssb[:, 128 * m:128 * (m + 1)],
                            a2t, start=True, stop=True,
                        )
                else:
                    for t in range(4):
                        nc.tensor.transpose(
                            sps[:, 256 * gl + 64 * t:256 * gl + 64 * (t + 1)],
                            src[p0:p0 + 64, 128 * t:128 * (t + 1)],
                            ident,
                        )
            if not CFG["ssb_split"]:
                drain(ssb[:], sps[:], _pick(CFG["ssb_engine"], pb))
                for m in range(4):
                    nc.tensor.matmul(
                        zps[:, 128 * m:128 * (m + 1)],
                        ssb[:, 128 * m:128 * (m + 1)],
                        a2t, start=True, stop=True,
                    )
            epi(zps, outsb, 1024 * (lb % SL) + 512 * pb, pb)
        if lb % SL == SL - 1:
            # out DRAM is partition-major; host un-permutes
            c0 = (lb - SL + 1) * 2 * 512
            if lb == NLB - 1 and CFG["split_last_store"]:
                for h in range(SL):
                    nc.sync.dma_start(
                        out_d[:, c0 + h * 1024:c0 + (h + 1) * 1024],
                        outsb[:, h * 1024:(h + 1) * 1024],
                    )
            else:
                nc.sync.dma_start(
                    out_d[:, c0:c0 + SL * 1024], outsb[:],
                )


def get_nc(variant="ln"):
    if variant not in _NC_CACHE:
        _NC_CACHE[variant] = _build_nc(variant=variant)
    return _NC_CACHE[variant]


def host_prep(inputs):
    adj = np.asarray(inputs["adj_weight"], np.float32)
    kern = np.ascontiguousarray(np.asarray(inputs["kernel"], np.float32))
    bias = np.asarray(inputs["bias"], np.float32)
    gamma = np.asarray(inputs["gamma"], np.float32)
    beta = np.asarray(inputs["beta"], np.float32)
    mm = np.asarray(inputs["moving_mean"], np.float32)
    mv = np.asarray(inputs["moving_var"], np.float32)

    deg = np.maximum(np.abs(adj).sum(axis=1, keepdims=True), 1e-8)
    dis = deg ** -0.5
    adj_hat = adj * dis * dis.T + np.eye(C, dtype=np.float32)
    a2t = np.zeros((128, 128), np.float32)
    a2t[:64, :64] = adj_hat.T
    a2t[64:, 64:] = adj_hat.T

    a = (gamma / np.sqrt(mv + BN_EPS)).astype(np.float32)
    b2 = (beta - mm * a).astype(np.float32)
    variant = "ln" if np.all(a > 0) else "safe"

    # kern laid out [128, j, d]: kern_sb[p, j, d] = kernel[128 j + p, d],
    # with the BN scale folded in on the ln path
    kern_f = kern * a[None, :] if variant == "ln" else kern
    kern_t = kern_f.reshape(4, 128, D).transpose(1, 0, 2).reshape(128, 4 * D)

    if CFG["direct_s1"]:
        cstb = np.zeros((128, 384), np.float32)
        cstb[:, 0:256] = kern_t
        cstb[:, 256:384] = a2t
    else:
        cstb = np.zeros((128, 512), np.float32)
        cstb[:, 0:128] = np.eye(128, dtype=np.float32)
        cstb[:, 128:384] = kern_t
        cstb[:, 384:512] = a2t
    cstb = to_bf16(cstb)

    # per-partition constant columns: d = partition % 64
    dd = np.arange(128) % 64
    cst2 = np.zeros((128, 8), np.float32)
    if variant == "ln":
        cst2[:, 0] = (1.0 / a)[dd]
        cst2[:, 1] = (bias + np.log(a))[dd]
        cst2[:, 2] = (a * bias + b2)[dd]
    else:
        cst2[:, 0] = 1.0
        cst2[:, 1] = bias[dd]
        cst2[:, 2] = bias[dd]
    cst2[:, 3] = a[dd]
    cst2[:, 4] = (b2 - a)[dd]
    cst2[:, 5] = b2[dd]

    x = np.asarray(inputs["x"], np.float32)
    shards = x.reshape(NCORES, R, Fdim)
    import ml_dtypes
    if CFG["xdt"] == "f8e3":
        def xprep(s):
            return np.ascontiguousarray(s.T).astype(ml_dtypes.float8_e3m4)
    else:
        def xprep(s):
            return np.ascontiguousarray(to_bf16(s).T).view(ml_dtypes.bfloat16)
    in_maps = [
        {
            "xs": xprep(shards[i]),
            "cstb": cstb.view(ml_dtypes.bfloat16),
            "cst2": cst2,
        }
        for i in range(NCORES)
    ]
    return in_maps, variant


def run(inputs, trace=False, **kw):
    in_maps, variant = host_prep(inputs)
    nc = get_nc(variant)
    try:
        res = bass_utils.run_bass_kernel_spmd(
            nc, in_maps, core_ids=list(range(NCORES)), trace=trace, **kw
        )
    except Exception:
        # transient NRT_EXEC_UNIT_UNRECOVERABLE has been observed right
        # after a previous process's teardown; one retry clears it
        import time as _time
        _time.sleep(5.0)
        res = bass_utils.run_bass_kernel_spmd(
            nc, in_maps, core_ids=list(range(NCORES)), trace=trace, **kw
        )
    shards = []
    for i in range(NCORES):
        raw = np.asarray(res.results[i]["out"]).astype(np.float32)
        # raw[p, C]: C = pbg*512 + 128*(2*gl+tq) + 64*h + c,
        # p = 64*ph + d; n = pbg*1024 + gl*512 + (2*tq+ph)*128 + 64*h + c
        shards.append(
            raw.reshape(2, 64, 16, 2, 2, 2, 64)
               .transpose(2, 3, 4, 0, 5, 6, 1)
               .reshape(R, D)
        )
    out = np.concatenate(shards, axis=0).reshape(B_FULL, C, D)
    return out, res


def kernel(**inputs) -> np.ndarray:
    out, _ = run(inputs)
    return out



# revision 65
# speedup vs baseline: 1.7655x; 1.0375x over previous
"""GCN message-passing kernel (nn_CARM_90185723281482) for 8 Trainium2 cores.

Computes, for x [2048, 64, 512], adj_weight [64, 64], kernel [512, 64]:
    adj_hat = D^-1/2 A D^-1/2 + I          (degree from row sums of |A|)
    out = BN(elu(adj_hat @ (x @ kernel) + bias))        -> [2048, 64, 64]

Sharding: data-parallel over the batch axis, 256 batches per core.
Per-core dataflow (rows n = (batch, channel) flattened, R = 16384 rows):
  - x ships fp8 e3m4 (4 mantissa bits), host-pre-transposed:
    xs[p, j, n] = x2d[n, 128 j + p].  Halves the dominant DMA stream vs
    bf16; measured end-to-end rel err 1.41e-2 (gate 2e-2).  e4m3 fails
    the gate (2.7e-2); e3m4's extra mantissa bit is what makes fp8 x
    viable.  The PE takes mixed fp8-stationary x bf16-moving operands.
  - DMA schedule (the kernel is memory-bound): ALL x loads are issued up
    front into a fully SBUF-resident copy (px pool holds all 8 MiB; no
    buffer reuse, so the load stream never blocks on compute), and ALL
    output stores are queued behind them on the SP queue, waiting on
    their epilogues.  The DMA device then runs [loads | stores] nearly
    back-to-back; only the last LB's epilogue chain can expose a gap.
  - stage 1 per 1024-row pb: support[n, d] = x_chunk.T @ kern per
    128-row group (x stationary), accumulating 4 f-chunks into a
    [128, 512] PSUM tile; stage-1 of all pbs of an LB run back-to-back
    on the PE, with each pb's PSUM->SBUF drain (ACT) hidden under the
    next pb's stage-1, and stage-2 + epilogue of LB k emitted after
    stage-1 of the NEXT pb (pb-granular software pipelining across the
    in-order PE queue, so a drain round-trip never stalls the PE).
  - stage 2 TRANSPOSED: z[d, n] = ssb_chunk.T @ a2t per 128-wide chunk,
    so d = partition % 64 — all BN/bias constants become per-partition
    scalars riding the ACT bias/scale ports and TensorScalarPtr operands
  - 3-op epilogue (a = gamma*rsqrt(var+eps) folded into the stage-1
    kernel when a > 0, so z_a = a*z comes off the PE; y = z + bias):
        q = exp(inv_a*z_a + bias + ln a)  = a*exp(y)           [ACT]
        t = min(q, a) + (b2 - a)                               [DVE ts]
        out = max(z_a + a*bias + b2, t)                        [DVE stt]
    which equals a*elu(y) + b2 on both branches (y <= e^y - 1 makes the
    max select the relu branch exactly when y >= 0).  The stt reads PSUM
    so it must stay on DVE (GPSIMD cannot access PSUM on hardware, even
    though the cost model accepts it).
  - last LB: loads split into 512/256-row pieces and drain/stage-2/
    epilogue split per 256-col half with SEPARATE PSUM banks per half
    (the cost model serializes a PE write and an ACT/DVE read of the
    same bank), so the final store's wait clears early enough for its
    descriptor generation to overlap the previous store's transfer.
  - output stored transposed [f, n]; host un-permutes.

Tuned against the TimelineSim cost model:
  64106 ns (session baseline, bf16 x) -> 56330 ns (DMA schedule)
  -> 36925 ns (fp8 x + pipeline rebalance) -> 35589 ns (pb-granular
  stage-2 lookahead + fine-path drain engine split).
  HW rel err 1.408e-2 (gate 2e-2).
"""

import sys

import numpy as np

sys.path.insert(0, "/opt/trn_rl_repo")

import concourse.bass as bass  # noqa: E402
from concourse import bacc, bass_utils, mybir, tile  # noqa: E402

F32 = mybir.dt.float32
BF16 = mybir.dt.bfloat16
FP8 = mybir.dt.float8e3  # e3m4: 4 mantissa bits, max 15.5
AF = mybir.ActivationFunctionType
OP = mybir.AluOpType

NCORES = 8
B_FULL, C, Fdim, D = 2048, 64, 512, 64
R = (B_FULL // NCORES) * C  # 16384 rows per core
LB_ROWS = 2048              # rows per load block
NLB = R // LB_ROWS          # 8 load blocks
BN_EPS = 1e-3

_NC_CACHE = {}

# Scheduling/balance knobs (tuned against the TimelineSim cost model)
CFG = {
    "deferred_stores": 1,  # all x loads first, all out stores after (SBUF-resident)
    "xdt": "f8e3",         # x wire/SBUF dtype: "bf16" | "f8e3" (halves DMA)
    "px": 8,
    "psT_sb": 8,
    "ps_sb": 5,
    "pep": 6,
    "psT_ps": 3,
    "ps_ps": 2,
    "po_ps": 5,
    "sT_split": 1,         # stage-1 PSUM: 0 = one [128,512], 1 = two [64,512]
    "ssb_split": 0,        # support drain: 0 = whole, 1 = per 256-col half
    "sT_engine": ["act"],          # stage-1 drain engine (per global pb)
    "ssb_engine": ["dve", "act"],  # support drain engine (legacy path)
    "r_engine": "act",     # relu (safe variant): "act" | "dve" | "pool"
    "t_engine": ["dve"] * 14 + ["gps", "gps"],  # min/add tensor_scalar
    # final scalar_tensor_tensor reads PSUM, so DVE only (GPSIMD cannot
    # access PSUM on hardware -- the cost model wrongly allows it)
    "add_engine": ["dve"],
    "epi_v2": 0,           # split linear branch (slower here; see notes)
    "r_engine2": ["gps"],  # v2 linear-branch engine
    "store_lbs": 4,
    "tailsplit": 1,
    "tailpiece": 512,      # last-LB load piece rows (deferred path)
    "tail256": 1,          # split the final load piece in two
    "fine2": 0,            # 3-window last pb races with tail256 on HW; keep halves
    "fine_lbs": 1,         # trailing LBs using fine drain/epilogue
    "interleave_pb": 1,    # coarse LBs: stage-1 of both pbs back-to-back
    "lb_rows": 2048,       # rows per load block (deferred path)
    "s2_depth": 1,         # LBs of lookahead before stage-2/epilogue
    "s2_depth_pb": 1,      # pb-granular lookahead (overrides s2_depth if >0)
    "drain_split": 0,      # coarse drains: halves on ACT+DVE in parallel
    "fillers": (0, 0, 0),  # PE keep-alive matmuls (tile scheduler hoists
                           # dependency-free work early, so these are inert)
    "load_lbs": 1,
    "split_last_store": 1,
    "split_store_lbs": 2,  # trailing LBs stored per-pb (half-width)
    "sT_fine_engine": ["dve", "act"],  # fine-path drain engine (2*(gkey%2)+wi)
    "epi_split_from": 16,  # gkey threshold for half-width coarse epilogues
    "fine_flush_mid": 0,   # flush pending stage-2 between the fine pbs
    "warmup_mm": 0,        # dummy matmuls (reading cstb) to ramp the PE
    "warmup_act": 0,       # dummy Exp to preload the ACT table early
    "head_pieces": (),
    "ep_split": 0,         # epilogue per 256-col half
    "direct_s1": 1,        # stage-1 with x stationary: no transposes/drains
    "d1_drainw": 512,      # direct stage-1 drain width (128|256|512)
}


def _pick(v, pb):
    """Engine knob: either a name or a [pb0, pb1] alternation list."""
    return v[pb % len(v)] if isinstance(v, (list, tuple)) else v


def to_bf16(a):
    """fp32 -> bf16 (RNE), returned as a uint16 array (raw bf16 bits)."""
    u = np.ascontiguousarray(a, np.float32).view(np.uint32).astype(np.uint64)
    r = (u + 0x7FFF + ((u >> 16) & 1)) >> 16
    return r.astype(np.uint16)


def _build_nc(loop_reps=None, variant="ln"):
    nc = bacc.Bacc(
        "TRN2", target_bir_lowering=False, debug=False, num_devices=NCORES
    )
    cstb_w = 384 if CFG["direct_s1"] else 512
    xdt = FP8 if CFG["xdt"] == "f8e3" else BF16
    xs_d = nc.dram_tensor("xs", [Fdim, R], xdt, kind="ExternalInput").ap()
    cstb_d = nc.dram_tensor("cstb", [128, cstb_w], BF16,
                            kind="ExternalInput").ap()
    cst2_d = nc.dram_tensor("cst2", [128, 8], F32, kind="ExternalInput").ap()
    out_d = nc.dram_tensor("out", [128, (R // 128) * D], BF16,
                           kind="ExternalOutput").ap()

    with tile.TileContext(nc) as tc, \
         tc.tile_pool(name="consts", bufs=1) as consts, \
         tc.tile_pool(name="px", bufs=CFG["px"]) as px, \
         tc.tile_pool(name="psT_ps", bufs=CFG["psT_ps"], space="PSUM") as psT_ps, \
         tc.tile_pool(name="psT_sb", bufs=CFG["psT_sb"]) as psT_sb, \
         tc.tile_pool(name="ps_ps", bufs=CFG["ps_ps"], space="PSUM") as ps_ps, \
         tc.tile_pool(name="ps_sb", bufs=CFG["ps_sb"]) as ps_sb, \
         tc.tile_pool(name="po_ps", bufs=CFG["po_ps"], space="PSUM") as po_ps, \
         tc.tile_pool(name="pep", bufs=CFG["pep"]) as pep, \
         tc.tile_pool(name="pwarm", bufs=1, space="PSUM") as pwarm, \
         tc.tile_pool(name="pout", bufs=2) as pout:

        cstb = consts.tile([128, cstb_w], BF16, tag="cstb")
        if not CFG["deferred_stores"]:
            nc.sync.dma_start(cstb[:], cstb_d)
        cst2 = consts.tile([128, 8], F32, tag="cst2")
        if CFG["direct_s1"]:
            identr = None
            kern = cstb[:, 0:256]
            a2t = cstb[:, 256:384]
        else:
            identr = cstb[:, 0:128]
            kern = cstb[:, 128:384]
            a2t = cstb[:, 384:512]
        inv_a = cst2[:, 0:1]
        bias_exp = cst2[:, 1:2]
        rbias = cst2[:, 2:3]
        a_col = cst2[:, 3:4]
        b3_col = cst2[:, 4:5]
        b2_col = cst2[:, 5:6]

        # PSUM->SBUF drains and element ops with an engine choice.
        def drain(dst_ap, src_ap, eng):
            if eng == "act":
                nc.scalar.activation(dst_ap, src_ap, AF.Copy)
            else:
                nc.vector.tensor_copy(dst_ap, src_ap)

        # Warmup: the PE runs at 0.65/1.2 GHz until ~3us of continuous
        # execution, and the first Activation pays a 1.3us table load.
        # Burn both on dummy reads of cstb while the first x pieces are
        # still in flight, so real work starts at full speed.
        if CFG["warmup_act"]:
            wact = consts.tile([128, 8], F32, tag="wact")
            nc.scalar.activation(wact[:], cstb[:, 0:8], AF.Exp)
        if CFG["warmup_mm"]:
            with tc.tile_pool(name="pwarm", bufs=1, space="PSUM") as pwarm:
                wps = pwarm.tile([64, 512], F32, tag="w")
                for _ in range(CFG["warmup_mm"]):
                    nc.tensor.matmul(wps[:], cstb[:, 0:64], cstb[:, 0:512],
                                     start=True, stop=True)

        import contextlib
        loop_cm = tc.For_i(0, loop_reps, 1) if loop_reps else \
            contextlib.nullcontext()
        with loop_cm:
            _body(nc, tc, locals(), variant)
    nc.compile()
    return nc


def _body(nc, tc, env, variant):
    px = env["px"]
    psT_ps, psT_sb = env["psT_ps"], env["psT_sb"]
    ps_ps, ps_sb, po_ps = env["ps_ps"], env["ps_sb"], env["po_ps"]
    pep, pout = env["pep"], env["pout"]
    xs_d, out_d, cst2_d = env["xs_d"], env["out_d"], env["cst2_d"]
    kern, identr, a2t = env["kern"], env["identr"], env["a2t"]
    inv_a, bias_exp, rbias = env["inv_a"], env["bias_exp"], env["rbias"]
    a_col, b3_col, b2_col = env["a_col"], env["b3_col"], env["b2_col"]
    cst2 = env["cst2"]
    cstb, cstb_d = env["cstb"], env["cstb_d"]
    xdt = env["xdt"]
    drain = env["drain"]

    def ts(eng, *a, **k):
        (nc.vector if eng == "dve" else nc.gpsimd).tensor_scalar(*a, **k)

    def tt_add(eng, out, x, y):
        if eng == "dve":
            nc.vector.tensor_add(out, x, y)
        else:
            nc.gpsimd.tensor_add(out, x, y)

    def epi_range(zps, outsb, ob, pb, c0, cw):
        # ln epilogue on cols [c0, c0+cw): q = a*exp(y), y = z+bias.
        # out = a*elu(y) + b2 = max(a*y + b2, min(q, a) + (b2-a))
        q = pep.tile([128, cw], BF16, tag=f"q{c0}_{cw}")
        t1 = pep.tile([128, cw], BF16, tag=f"t{c0}_{cw}")
        nc.scalar.activation(q[:], zps[:, c0:c0 + cw], AF.Exp,
                             bias=bias_exp, scale=inv_a)
        if CFG["epi_v2"]:
            # Both PSUM readers (q, r) fire right after stage-2, so the
            # zps bank recycles ~1.2us sooner — without this, stage-2 of
            # pb k+po_ps stalls on the slow final op via the WAR ring.
            r = pep.tile([128, cw], BF16, tag=f"r{c0}_{cw}")
            r_eng = _pick(CFG["r_engine2"], pb)
            if r_eng == "act":
                nc.scalar.activation(r[:], zps[:, c0:c0 + cw], AF.Copy,
                                     bias=rbias)
            else:
                ts(r_eng, r[:], zps[:, c0:c0 + cw],
                   rbias, 0.0, OP.add, OP.bypass)
            ts(_pick(CFG["t_engine"], pb), t1[:], q[:], a_col, b3_col,
               OP.min, OP.add)
            f_eng = _pick(CFG["add_engine"], pb)
            (nc.vector if f_eng == "dve" else
             nc.gpsimd).scalar_tensor_tensor(
                outsb[:, ob + c0:ob + c0 + cw], r[:], 0.0,
                t1[:], OP.add, OP.max)
            return
        ts(_pick(CFG["t_engine"], pb), t1[:], q[:], a_col, b3_col,
           OP.min, OP.add)
        stt_eng = _pick(CFG["add_engine"], pb)
        (nc.vector if stt_eng == "dve" else
         nc.gpsimd).scalar_tensor_tensor(
            outsb[:, ob + c0:ob + c0 + cw], zps[:, c0:c0 + cw], rbias,
            t1[:], OP.add, OP.max)

    def epi(zps, outsb, ob, pb):
        # epilogue: per-partition constants (d = partition % 64)
        if variant == "ln":
            epi_range(zps, outsb, ob, pb, 0, 512)
        else:
            q = pep.tile([128, 512], BF16, tag="q")
            nc.scalar.activation(q[:], zps[:], AF.Exp,
                                 bias=bias_exp, scale=inv_a)
            r = pep.tile([128, 512], BF16, tag="r")
            if _pick(CFG["r_engine"], pb) == "act":
                nc.scalar.activation(r[:], zps[:], AF.Relu, bias=rbias)
            else:
                ts(_pick(CFG["r_engine"], pb), r[:], zps[:], rbias, 0.0,
                   OP.add, OP.max)
            # safe for a<=0: q=exp(z+bias), r=relu(z+bias);
            # elu = r + min(q-1, 0); out = a*elu + b2
            t1 = pep.tile([128, 512], BF16, tag="t")
            ts(_pick(CFG["t_engine"], pb), t1[:], q[:], 1.0, 0.0,
               OP.subtract, OP.min)
            s1 = pep.tile([128, 512], BF16, tag="s")
            tt_add(_pick(CFG["add_engine"], pb), s1[:], t1[:], r[:])
            ts("dve", outsb[:, ob:ob + 512], s1[:], a_col, b2_col,
               OP.mult, OP.add)

    xsT_v = xs_d.rearrange("(j p) n -> p j n", p=128)
    LL = CFG["load_lbs"]
    SL = CFG["store_lbs"]

    def direct_pb(xsb_v, nw0, pb, outsb, ob, fine=False, fill_mid=0,
                  fill=None, gkey=None):
        # stage 1 direct: support[n, d] = xT_chunk.T @ kern per 128-row
        # group (x stationary, kern moving); groups (2m, 2m+1) land side
        # by side as stage-2 chunk m.  fine=True pipelines drain/stage-2/
        # epilogue per 256-col half to shrink the end-of-kernel chain.
        if gkey is None:
            gkey = pb
        ssb = ps_sb.tile([128, 512], BF16, tag="ss")
        zps = po_ps.tile([128, 512], F32, tag="op")
        sp = psT_ps.tile([128, 512], F32, tag="sTp")
        dw = 256 if fine else CFG["d1_drainw"]

        def s1_group(m, tgt):
            for g2 in range(2):
                g8 = 2 * m + g2
                r0 = nw0 + 1024 * pb + 128 * g8
                oc = 128 * m + 64 * g2
                for j in range(4):
                    nc.tensor.matmul(
                        tgt[:, oc:oc + 64],
                        xsb_v[:, j, r0:r0 + 128],
                        kern[:, 64 * j:64 * (j + 1)],
                        start=(j == 0),
                        stop=(j == 3),
                    )

        def s2_chunks(c0, c1, z):
            for mm in range(c0 // 128, c1 // 128):
                nc.tensor.matmul(
                    z[:, 128 * mm:128 * (mm + 1)],
                    ssb[:, 128 * mm:128 * (mm + 1)],
                    a2t, start=True, stop=True,
                )

        if fine and variant == "ln":
            # Tail chain per window.  Each window gets its OWN PSUM banks:
            # the cost model serializes a PE write and an ACT/DVE read of
            # the same PSUM bank, so sharing banks between windows would
            # stall the next window's stage-1 behind this window's drain
            # and its stage-2 behind this window's exp.  fine==2 puts the
            # last 128 cols in their own window so the final load piece
            # feeds a minimal chain.
            wins = [(0, 2), (2, 3), (3, 4)] if fine == 2 else \
                [(0, 2), (2, 4)]
            for wi, (m0, m1) in enumerate(wins):
                tgt = sp if wi == 0 else psT_ps.tile([128, 512], F32,
                                                     tag="sTp")
                z = zps if wi == 0 else po_ps.tile([128, 512], F32,
                                                   tag="op")
                for m in range(m0, m1):
                    s1_group(m, tgt)
                c0, c1 = 128 * m0, 128 * m1
                drain(ssb[:, c0:c1], tgt[:, c0:c1],
                      _pick(CFG["sT_fine_engine"], 2 * (gkey % 2) + wi))
                s2_chunks(c0, c1, z)
                epi_range(z, outsb, ob, gkey, c0, c1 - c0)
            return

        for m in range(4):
            if m == 2 and fill_mid:
                fill(fill_mid)
            s1_group(m, sp)
            c1 = 128 * (m + 1)
            if c1 % dw == 0:
                c0 = c1 - dw
                drain(ssb[:, c0:c1], sp[:, c0:c1],
                      _pick(CFG["sT_engine"], 2 * gkey + m))
                s2_chunks(c0, c1, zps)
        epi(zps, outsb, ob, gkey)

    def direct_lb_s1(xsb_v, lb, npb, OW):
        # Stage-1 + drain for every pb of an LB, stage-1 back-to-back on
        # the PE so a pb's drain round-trip (PE -> ACT/DVE -> PE) hides
        # under the next pb's stage-1.  Separate PSUM banks per pb.
        # Stage-2/epilogue are emitted by the caller s2_depth LBs later,
        # so the drain never head-of-line-blocks a later LB's stage-1 on
        # the in-order PE queue.
        parts = []
        for pb in range(npb):
            parts.append(direct_pb_s1(xsb_v, lb, pb, OW))
        return parts

    def direct_pb_s1(xsb_v, lb, pb, OW):
        npb = CFG["lb_rows"] // 1024
        gkey = npb * lb + pb
        ssb = ps_sb.tile([128, 512], BF16, tag="ss")
        sp = psT_ps.tile([128, 512], F32, tag="sTp")

        for m in range(4):
            for g2 in range(2):
                g8 = 2 * m + g2
                r0 = 1024 * pb + 128 * g8
                oc = 128 * m + 64 * g2
                for j in range(4):
                    nc.tensor.matmul(
                        sp[:, oc:oc + 64],
                        xsb_v[:, j, r0:r0 + 128],
                        kern[:, 64 * j:64 * (j + 1)],
                        start=(j == 0),
                        stop=(j == 3),
                    )
        if CFG["drain_split"]:
            e0 = _pick(CFG["sT_engine"], gkey)
            e1 = "dve" if e0 == "act" else "act"
            drain(ssb[:, 0:256], sp[:, 0:256], e0)
            drain(ssb[:, 256:512], sp[:, 256:512], e1)
        else:
            drain(ssb[:], sp[:], _pick(CFG["sT_engine"], gkey))
        return (OW * lb + 512 * pb, gkey, ssb)

    def direct_lb_s2(parts, outsb):
        for ob, gkey, ssb in parts:
            zps = po_ps.tile([128, 512], F32, tag="op")
            for mm in range(4):
                nc.tensor.matmul(
                    zps[:, 128 * mm:128 * (mm + 1)],
                    ssb[:, 128 * mm:128 * (mm + 1)],
                    a2t, start=True, stop=True,
                )
            if variant == "ln" and gkey >= CFG["epi_split_from"]:
                # trailing LBs: half-width epilogues so each half-store's
                # wait clears as soon as its own stt lands
                epi_range(zps, outsb, ob, gkey, 0, 256)
                epi_range(zps, outsb, ob, gkey, 256, 256)
            else:
                epi(zps, outsb, ob, gkey)

    if CFG["deferred_stores"]:
        assert CFG["direct_s1"]
        LBR = CFG["lb_rows"]
        nlb = R // LBR
        npb = LBR // 1024
        OW = LBR // 2
        # Phase 1: issue every x load up front (no buffer reuse, so the
        # load stream never waits on compute).
        xsb_views = []
        for lb in range(nlb):
            xsb = px.tile([128, 4 * LBR], xdt, tag="x")
            xsb_v = xsb[:].rearrange("p (j n) -> p j n", j=4)
            xsb_views.append(xsb_v)
            if lb == 0 and CFG["head_pieces"]:
                pieces = list(CFG["head_pieces"])
                rest = LBR - sum(pieces)
                pieces += [rest] if rest else []
            elif lb == nlb - 1 and CFG["tailsplit"]:
                tp = CFG["tailpiece"]
                pieces = [tp] * (LBR // tp)
                if CFG["tail256"] and tp >= 512:
                    # split the final piece so the last-arriving data
                    # feeds only the m3 group of the last pb
                    pieces = pieces[:-1] + [tp // 2] * 2
            else:
                pieces = [LBR]
            n0 = 0
            for pi, pn in enumerate(pieces):
                nc.sync.dma_start(
                    xsb_v[:, :, n0:n0 + pn],
                    xsT_v[:, :, lb * LBR + n0:lb * LBR + n0 + pn],
                )
                n0 += pn
                if lb == 0 and pi == 0:
                    # constants ride behind the first x piece: their DGE
                    # overlaps its (long) transfer, so no stream gap
                    nc.sync.dma_start(cstb[:], cstb_d)
                    nc.sync.dma_start(cst2[:], cst2_d)
        # Phase 2: compute, all epilogues land in one persistent outsb.
        outsb = pout.tile([128, R // 2], BF16, tag="out")
        D = CFG["s2_depth"]
        pend = []
        for lb in range(nlb):
            xsb_v = xsb_views[lb]
            if CFG["interleave_pb"] and lb < nlb - CFG["fine_lbs"]:
                if CFG["s2_depth_pb"]:
                    for pb in range(npb):
                        pend.append([direct_pb_s1(xsb_v, lb, pb, OW)])
                        while len(pend) > CFG["s2_depth_pb"]:
                            direct_lb_s2(pend.pop(0), outsb)
                else:
                    pend.append(direct_lb_s1(xsb_v, lb, npb, OW))
                    if len(pend) > D:
                        direct_lb_s2(pend.pop(0), outsb)
                continue
            last = lb == nlb - 1
            if not CFG["fine_flush_mid"]:
                while pend:
                    direct_lb_s2(pend.pop(0), outsb)
            for pb in range(npb):
                gkey = npb * lb + pb
                fine = 2 if (last and pb == npb - 1 and CFG["fine2"]) else 1
                direct_pb(xsb_v, 0, pb, outsb, OW * lb + 512 * pb,
                          fine=fine, gkey=gkey)
                if CFG["fine_flush_mid"] and pb == 0:
                    # previous LB's stage-2/epilogue rides between the two
                    # fine pbs so the fine stage-1 isn't queued behind it
                    while pend:
                        direct_lb_s2(pend.pop(0), outsb)
        while pend:
            direct_lb_s2(pend.pop(0), outsb)
        # Phase 3: stores, queued behind every load on the SP queue.
        # Per-LB granularity; the final LB split in two so the last
        # transfer (and its post-DMA sem prop) is small and late-arriving
        # epilogues can't stall much ahead of it.
        half = OW // 2
        for lb in range(nlb):
            if lb >= nlb - CFG["split_store_lbs"]:
                for h in range(2):
                    c0 = OW * lb + half * h
                    nc.sync.dma_start(out_d[:, c0:c0 + half],
                                      outsb[:, c0:c0 + half])
            else:
                nc.sync.dma_start(
                    out_d[:, OW * lb:OW * (lb + 1)],
                    outsb[:, OW * lb:OW * (lb + 1)],
                )
        return

    for lb in range(NLB):
        if lb % LL == 0:
            xsb = px.tile([128, 4 * LL * LB_ROWS], xdt, tag="x")
            xsb_v = xsb[:].rearrange("p (j n) -> p j n", j=4)
            # Split the first/last loads so compute starts early
            if lb == 0:
                pieces = list(CFG["head_pieces"])
                rest = LL * LB_ROWS - sum(pieces)
                pieces += [rest] if rest else []
            elif lb == NLB - LL and CFG["tailsplit"]:
                pieces = [1024] * (LL * LB_ROWS // 1024)
            else:
                pieces = [LL * LB_ROWS]
            n0 = 0
            for pi, pn in enumerate(pieces):
                nc.sync.dma_start(
                    xsb_v[:, :, n0:n0 + pn],
                    xsT_v[:, :, lb * LB_ROWS + n0:lb * LB_ROWS + n0 + pn],
                )
                n0 += pn
                if lb == 0 and pi == 0:
                    # tiny f32 constant columns; issued after the first x
                    # piece so they don't delay the pipeline start
                    nc.sync.dma_start(cst2[:], cst2_d)
        nw0 = (lb % LL) * LB_ROWS
        if lb % SL == 0:
            outsb = pout.tile([128, SL * 2 * 512], BF16, tag="out")
        for pb in range(2):
            if CFG["direct_s1"]:
                # stage 1 direct: support[n, d] = xT_chunk.T @ kern per
                # 128-row group (x stationary, kern moving) — no PE
                # transposes, no supportT drains. Groups (2m, 2m+1) land
                # side by side as stage-2 chunk m.
                ssb = ps_sb.tile([128, 512], BF16, tag="ss")
                zps = po_ps.tile([128, 512], F32, tag="op")
                sp = psT_ps.tile([128, 512], F32, tag="sTp")
                dw = CFG["d1_drainw"]
                for m in range(4):
                    for g2 in range(2):
                        g8 = 2 * m + g2
                        r0 = nw0 + 1024 * pb + 128 * g8
                        oc = 128 * m + 64 * g2
                        for j in range(4):
                            nc.tensor.matmul(
                                sp[:, oc:oc + 64],
                                xsb_v[:, j, r0:r0 + 128],
                                kern[:, 64 * j:64 * (j + 1)],
                                start=(j == 0),
                                stop=(j == 3),
                            )
                    c1 = 128 * (m + 1)
                    if c1 % dw == 0:
                        c0 = c1 - dw
                        drain(ssb[:, c0:c1], sp[:, c0:c1],
                              _pick(CFG["sT_engine"], 2 * pb + m))
                        for mm in range(c0 // 128, c1 // 128):
                            nc.tensor.matmul(
                                zps[:, 128 * mm:128 * (mm + 1)],
                                ssb[:, 128 * mm:128 * (mm + 1)],
                                a2t, start=True, stop=True,
                            )
                epi(zps, outsb, 1024 * (lb % SL) + 512 * pb, pb)
                continue
            # stage 1: supportT [d, n]; one [128,512] tile or two [64,512]
            if CFG["sT_split"]:
                sT_views = []
                for gl in range(2):
                    g = 2 * pb + gl
                    sTps = psT_ps.tile([64, 512], F32, tag="sTp")
                    for j in range(4):
                        nc.tensor.matmul(
                            sTps[:],
                            kern[:, 64 * j:64 * (j + 1)],
                            xsb_v[:, j, nw0 + 512 * g:nw0 + 512 * (g + 1)],
                            start=(j == 0),
                            stop=(j == 3),
                        )
                    sTsb = psT_sb.tile([64, 512], BF16, tag="sTs")
                    drain(sTsb[:], sTps[:],
                          _pick(CFG["sT_engine"], 2 * pb + gl))
                    sT_views.append((sTsb, 0))
            else:
                sTps = psT_ps.tile([128, 512], F32, tag="sTp")
                for gl in range(2):
                    g = 2 * pb + gl
                    for j in range(4):
                        nc.tensor.matmul(
                            sTps[64 * gl:64 * (gl + 1), :],
                            kern[:, 64 * j:64 * (j + 1)],
                            xsb_v[:, j, nw0 + 512 * g:nw0 + 512 * (g + 1)],
                            start=(j == 0),
                            stop=(j == 3),
                        )
                sTsb = psT_sb.tile([128, 512], BF16, tag="sTs")
                drain(sTsb[:], sTps[:], _pick(CFG["sT_engine"], pb))
                sT_views = [(sTsb, 0), (sTsb, 64)]
            # transpose supportT -> support chunks [n, (gl,t,d)], then
            # drain + stage 2 per half so halves pipeline
            ssb = ps_sb.tile([128, 512], BF16, tag="ss")
            zps = po_ps.tile([128, 512], F32, tag="op")
            if not CFG["ssb_split"]:
                sps = ps_ps.tile([128, 512], BF16, tag="sp")
            for gl in range(2):
                src, p0 = sT_views[gl]
                ident = identr[p0:p0 + 64, p0:p0 + 64] if p0 else \
                    identr[:64, :64]
                if CFG["ssb_split"]:
                    # per-gl [128,256] PSUM tile: half the bank footprint
                    sps_g = ps_ps.tile([128, 256], BF16, tag="sp")
                    for t in range(4):
                        nc.tensor.transpose(
                            sps_g[:, 64 * t:64 * (t + 1)],
                            src[p0:p0 + 64, 128 * t:128 * (t + 1)],
                            ident,
                        )
                    h0 = 256 * gl
                    drain(ssb[:, h0:h0 + 256], sps_g[:],
                          _pick(CFG["ssb_engine"], 2 * pb + gl))
                    for m in (2 * gl, 2 * gl + 1):
                        nc.tensor.matmul(
                            zps[:, 128 * m:128 * (m + 1)],
                            ssb[:, 128 * m:128 * (m + 1)],
                            a2t, start=True, stop=True,
                        )
                else:
                    for t in range(4):
                        nc.tensor.transpose(
                            sps[:, 256 * gl + 64 * t:256 * gl + 64 * (t + 1)],
                            src[p0:p0 + 64, 128 * t:128 * (t + 1)],
                            ident,
                        )
            if not CFG["ssb_split"]:
                drain(ssb[:], sps[:], _pick(CFG["ssb_engine"], pb))
                for m in range(4):
                    nc.tensor.matmul(
                        zps[:, 128 * m:128 * (m + 1)],
                        ssb[:, 128 * m:128 * (m + 1)],
                        a2t, start=True, stop=True,
                    )
            epi(zps, outsb, 1024 * (lb % SL) + 512 * pb, pb)
        if lb % SL == SL - 1:
            # out DRAM is partition-major; host un-permutes
            c0 = (lb - SL + 1) * 2 * 512
            if lb == NLB - 1 and CFG["split_last_store"]:
                for h in range(SL):
                    nc.sync.dma_start(
                        out_d[:, c0 + h * 1024:c0 + (h + 1) * 1024],
                        outsb[:, h * 1024:(h + 1) * 1024],
                    )
            else:
                nc.sync.dma_start(
                    out_d[:, c0:c0 + SL * 1024], outsb[:],
                )


def get_nc(variant="ln"):
    if variant not in _NC_CACHE:
        _NC_CACHE[variant] = _build_nc(variant=variant)
    return _NC_CACHE[variant]


def host_prep(inputs):
    adj = np.asarray(inputs["adj_weight"], np.float32)
    kern = np.ascontiguousarray(np.asarray(inputs["kernel"], np.float32))
    bias = np.asarray(inputs["bias"], np.float32)
    gamma = np.asarray(inputs["gamma"], np.float32)
    beta = np.asarray(inputs["beta"], np.float32)
    mm = np.asarray(inputs["moving_mean"], np.float32)
    mv = np.asarray(inputs["moving_var"], np.float32)

    deg = np.maximum(np.abs(adj).sum(axis=1, keepdims=True), 1e-8)
    dis = deg ** -0.5
    adj_hat = adj * dis * dis.T + np.eye(C, dtype=np.float32)
    a2t = np.zeros((128, 128), np.float32)
    a2t[:64, :64] = adj_hat.T
    a2t[64:, 64:] = adj_hat.T

    a = (gamma / np.sqrt(mv + BN_EPS)).astype(np.float32)
    b2 = (beta - mm * a).astype(np.float32)
    variant = "ln" if np.all(a > 0) else "safe"

    # kern laid out [128, j, d]: kern_sb[p, j, d] = kernel[128 j + p, d],
    # with the BN scale folded in on the ln path
    kern_f = kern * a[None, :] if variant == "ln" else kern
    kern_t = kern_f.reshape(4, 128, D).transpose(1, 0, 2).reshape(128, 4 * D)

    if CFG["direct_s1"]:
        cstb = np.zeros((128, 384), np.float32)
        cstb[:, 0:256] = kern_t
        cstb[:, 256:384] = a2t
    else:
        cstb = np.zeros((128, 512), np.float32)
        cstb[:, 0:128] = np.eye(128, dtype=np.float32)
        cstb[:, 128:384] = kern_t
        cstb[:, 384:512] = a2t
    cstb = to_bf16(cstb)

    # per-partition constant columns: d = partition % 64
    dd = np.arange(128) % 64
    cst2 = np.zeros((128, 8), np.float32)
    if variant == "ln":
        cst2[:, 0] = (1.0 / a)[dd]
        cst2[:, 1] = (bias + np.log(a))[dd]
        cst2[:, 2] = (a * bias + b2)[dd]
    else:
        cst2[:, 0] = 1.0
        cst2[:, 1] = bias[dd]
        cst2[:, 2] = bias[dd]
    cst2[:, 3] = a[dd]
    cst2[:, 4] = (b2 - a)[dd]
    cst2[:, 5] = b2[dd]

    x = np.asarray(inputs["x"], np.float32)
    shards = x.reshape(NCORES, R, Fdim)
    import ml_dtypes
    if CFG["xdt"] == "f8e3":
        def xprep(s):
            return np.ascontiguousarray(s.T).astype(ml_dtypes.float8_e3m4)
    else:
        def xprep(s):
            return np.ascontiguousarray(to_bf16(s).T).view(ml_dtypes.bfloat16)
    in_maps = [
        {
            "xs": xprep(shards[i]),
            "cstb": cstb.view(ml_dtypes.bfloat16),
            "cst2": cst2,
        }
        for i in range(NCORES)
    ]
    return in_maps, variant


def run(inputs, trace=False, **kw):
    in_maps, variant = host_prep(inputs)
    nc = get_nc(variant)
    try:
        res = bass_utils.run_bass_kernel_spmd(
            nc, in_maps, core_ids=list(range(NCORES)), trace=trace, **kw
        )
    except Exception:
        # transient NRT_EXEC_UNIT_UNRECOVERABLE has been observed right
        # after a previous process's teardown; one retry clears it
        import time as _time
        _time.sleep(5.0)
        res = bass_utils.run_bass_kernel_spmd(
            nc, in_maps, core_ids=list(range(NCORES)), trace=trace, **kw
        )
    shards = []
    for i in range(NCORES):
        raw = np.asarray(res.results[i]["out"]).astype(np.float32)
        # raw[p, C]: C = pbg*512 + 128*(2*gl+tq) + 64*h + c,
        # p = 64*ph + d; n = pbg*1024 + gl*512 + (2*tq+ph)*128 + 64*h + c
        shards.append(
            raw.reshape(2, 64, 16, 2, 2, 2, 64)
               .transpose(2, 3, 4, 0, 5, 6, 1)
               .reshape(R, D)
        )
    out = np.concatenate(shards, axis=0).reshape(B_FULL, C, D)
    return out, res


def kernel(**inputs) -> np.ndarray:
    out, _ = run(inputs)
    return out

